# revision 15
# baseline (speedup 1.0000x reference)
"""TRN2 Bass kernel for nn_DeepRoute (pointer-network greedy decoder).

Self-contained: hardcodes shapes, shards batch over 8 NeuronCores,
runs one SPMD Bass program per core, gathers full outputs.

Layouts (per core, b = 64 local batch rows):
  packed attention tiles: partitions r = hh*64 + b (hh = h-half), free:
    E_gl/E_ptr: (l, h') "lh";  F: (h', l) "hl"
  LSTM state: c [64(b), 256(h)]; h kept as h2 = 2*h in [64, 256] and
  transposed hT [128(h'), (k,b)]; weights host-pre-scaled to absorb the 2x
  (sigmoid computed as 0.5*(1+tanh(x/2)) -- only tanh/exp ACT tables used).
"""
import sys, os
sys.path.insert(0, "/opt/trn_rl_repo")
import numpy as np

L, B, E, H, NC = 32, 512, 256, 256, 8
BL = B // NC          # 64 batch rows per core
C_TANH = 10.0
NEG = np.float32(-np.inf)

_PROG = None  # cached compiled Bacc program


# ----------------------------------------------------------------------------
# program builder
# ----------------------------------------------------------------------------
def _build():
    import concourse.bass as bass
    import concourse.mybir as mybir
    import concourse.tile as tile
    from concourse import bacc

    F32, I32 = mybir.dt.float32, mybir.dt.int32

    nc = bacc.Bacc("TRN2", target_bir_lowering=False, debug=False,
                   enable_asserts=False)

    d = {}
    def din(name, shape, dt=F32):
        d[name] = nc.dram_tensor(name, shape, dt, kind="ExternalInput").ap()
        return d[name]

    # per-core inputs (host-packed layouts; see _prep_core)
    din("xT0", [128, 128]); din("h0T", [128, 128]); din("c0", [64, 256])
    din("ctxTd", [128, 4096])
    din("W_ihT", [128, 2048]); din("W_hhT", [128, 2048])
    din("glWqT", [128, 512]); din("ptrWqT", [128, 512])
    din("glWrT", [128, 512]); din("ptrWrT", [128, 512]); din("glWr_raw", [128, 512])
    din("bias_ih", [1, 1024]); din("bias_hh", [1, 1024])
    din("glbq", [128, 2]); din("glbr", [128, 2])
    din("ptrbq", [128, 2]); din("ptrbr", [128, 2])
    din("glv_rep", [128, 128]); din("ptrv_rep", [128, 128])
    din("ones64", [1, 64]); din("ident", [128, 128])
    din("iotaL", [128, 32]); din("iotaB32", [64, 1], I32)
    din("minf", [128, 32]); din("zeros1", [128, 1])
    din("lnmask0", [128, 32])
    din("embBL", [BL * L, 256])

    logp_o = nc.dram_tensor("logp_o", [BL, L * L], F32, kind="ExternalOutput").ap()
    sels_o = nc.dram_tensor("sels_o", [BL, L], I32, kind="ExternalOutput").ap()

    with tile.TileContext(nc) as tc:
        _emit(tc, nc, d, logp_o, sels_o, bass, mybir)
    nc.compile()
    return nc


def _emit(tc, nc, d, logp_o, sels_o, bass, mybir):
    F32, I32, U32 = mybir.dt.float32, mybir.dt.int32, mybir.dt.uint32
    AF, OP, AX = (mybir.ActivationFunctionType, mybir.AluOpType,
                  mybir.AxisListType)
    NCH = 4                      # chunks per big DVE op
    CW = 4096 // NCH             # chunk width (cols)
    LC = 32 // NCH               # l per chunk (lh layout)
    HC = 128 // NCH              # h' per chunk (hl layout)

    from contextlib import ExitStack
    with ExitStack() as ctx:
        pp = ctx.enter_context(tc.tile_pool(name="pp", bufs=1))
        st = ctx.enter_context(tc.tile_pool(name="st", bufs=2))

        # ---------- persistent SBUF ----------
        def load(pool, name, shape, dt=F32):
            t = pool.tile(shape, dt, tag=name, name=f"sb_{name}")
            nc.sync.dma_start(t[:], d[name][:])
            return t

        W_ihT = load(pp, "W_ihT", [128, 2048])
        W_hhT = load(pp, "W_hhT", [128, 2048])
        glWqT = load(pp, "glWqT", [128, 512])
        glv = load(pp, "glv_rep", [128, 128]); ptrv = load(pp, "ptrv_rep", [128, 128])
        ones64 = load(pp, "ones64", [1, 64]); ident = load(pp, "ident", [128, 128])
        iotaL = load(pp, "iotaL", [128, 32]); iotaB32 = load(pp, "iotaB32", [64, 1], I32)
        minf = load(pp, "minf", [128, 32]); zeros1 = load(pp, "zeros1", [128, 1])

        lnmask = pp.tile([128, 32], F32, tag="lnmask", name="lnmask")
        nc.sync.dma_start(lnmask[:], d["lnmask0"][:])

        E_gl = pp.tile([128, 4096], F32, tag="E_gl", name="E_gl")     # (l, h')
        E_ptr = pp.tile([128, 4096], F32, tag="E_ptr", name="E_ptr")  # (l, h')
        F_p = pp.tile([128, 4096], F32, tag="F_p", name="F_p")        # (h', l)
        logp_buf = pp.tile([64, 1024], F32, tag="logp_buf", name="logp_buf")
        logp_fin = pp.tile([64, 1024], F32, tag="logp_fin", name="logp_fin")
        nmax_buf = pp.tile([64, 32], F32, tag="nmax_buf", name="nmax_buf")
        s2_buf = pp.tile([64, 32], F32, tag="s2_buf", name="s2_buf")
        sels_f = pp.tile([64, 32], F32, tag="sels_f", name="sels_f")
        bias_row = pp.tile([1, 1024], F32, tag="bias_row", name="bias_row")

        # ---------- precompute: E_gl, E_ptr, F (one-time) ----------
        with tc.tile_pool(name="pre", bufs=1) as pre, \
             tc.tile_pool(name="preps", bufs=1, space="PSUM") as preps:
            ctxT = load(pre, "ctxTd", [128, 4096])
            glWrT = load(pre, "glWrT", [128, 512])
            ptrWrT = load(pre, "ptrWrT", [128, 512])
            ptrWqT = load(pre, "ptrWqT", [128, 512])
            glWr_raw = load(pre, "glWr_raw", [128, 512])
            b_ih = load(pre, "bias_ih", [1, 1024])
            b_hh = load(pre, "bias_hh", [1, 1024])
            glbq = load(pre, "glbq", [128, 2]); glbr = load(pre, "glbr", [128, 2])
            ptrbq = load(pre, "ptrbq", [128, 2]); ptrbr = load(pre, "ptrbr", [128, 2])

            nc.vector.tensor_tensor(bias_row[:], b_ih[:], b_hh[:], op=OP.add)
            bias_glE = pre.tile([128, 2], F32, tag="bias_glE", name="bias_glE")
            nc.vector.tensor_tensor(bias_glE[:], glbr[:], glbq[:], op=OP.add)
            bias_ptrE = pre.tile([128, 2], F32, tag="bias_ptrE", name="bias_ptrE")
            nc.vector.tensor_tensor(bias_ptrE[:], ptrbr[:], ptrbq[:], op=OP.add)

            # WfT[e, o] = sum_h gl_Wr[h, e] * ptr_Wq.T[h, o]  (for F = ctx@Wf.T)
            WfT = pre.tile([128, 512], F32, tag="WfT", name="WfT")
            for m in range(2):     # e-half
                wps = preps.tile([128, 256], F32, tag="wps", name=f"wps{m}")
                for k in range(2):  # h-half
                    nc.tensor.matmul(
                        wps[:], glWr_raw[:, k * 256 + m * 128:k * 256 + m * 128 + 128],
                        ptrWqT[:, k * 256:(k + 1) * 256],
                        start=(k == 0), stop=(k == 1))
                nc.vector.tensor_copy(WfT[:, m * 256:(m + 1) * 256], wps[:])

            # bias for F: bf = ptr_Wq @ gl_br
            bfp = preps.tile([128, 2], F32, tag="bfp", name="bfp")
            for hh in range(2):
                for k in range(2):
                    nc.tensor.matmul(bfp[:, hh:hh + 1],
                                     ptrWqT[:, k * 256 + hh * 128:k * 256 + hh * 128 + 128],
                                     glbr[:, k:k + 1],
                                     start=(k == 0), stop=(k == 1))
            bias_F = pre.tile([128, 2], F32, tag="bias_F", name="bias_F")
            nc.vector.tensor_copy(bias_F[:], bfp[:])

            # stream each big tensor: mm chunk -> stage (+bias) -> transpose-repack
            def emit_packed(lhsT, biasE, dst, hl, name):
                for hh in range(2):          # output h-half
                    eps = preps.tile([128, 2048], F32, tag="eps", name=f"eps_{name}{hh}")
                    for c in range(4):
                        for k in range(2):
                            nc.tensor.matmul(
                                eps[:, c * 512:(c + 1) * 512],
                                lhsT[:, k * 256 + hh * 128:k * 256 + hh * 128 + 128],
                                ctxT[:, k * 2048 + c * 512:k * 2048 + (c + 1) * 512],
                                start=(k == 0), stop=(k == 1))
                    stage = pre.tile([128, 2048], F32, tag="stage", name=f"stg_{name}{hh}")
                    nc.vector.tensor_scalar(stage[:], in0=eps[:],
                                            scalar1=biasE[:, hh:hh + 1],
                                            scalar2=0.0, op0=OP.add)
                    st3 = stage[:].rearrange("p (b l) -> p b l", l=32)
                    for l in range(32):
                        tp = preps.tile([64, 128], F32, tag="tp", name=f"tp_{name}{hh}_{l}")
                        nc.tensor.transpose(tp[:], st3[:, :, l], ident[:])
                        if hl:
                            dstv = dst[hh * 64:(hh + 1) * 64, :] \
                                .rearrange("p (h l) -> p h l", l=32)[:, :, l]
                        else:
                            dstv = dst[hh * 64:(hh + 1) * 64, l * 128:(l + 1) * 128]
                        nc.vector.tensor_copy(dstv, tp[:])

            emit_packed(glWrT, bias_glE, E_gl, False, "gl")
            emit_packed(ptrWrT, bias_ptrE, E_ptr, False, "ptr")
            emit_packed(WfT, bias_F, F_p, True, "F")

        # ---------- working pools (opened after precompute frees space) ----------
        wk = ctx.enter_context(tc.tile_pool(name="wk", bufs=2))
        wkc = ctx.enter_context(tc.tile_pool(name="wkc", bufs=4))
        ps = ctx.enter_context(tc.tile_pool(name="ps", bufs=1, space="PSUM"))

        cur_x = st.tile([128, 128], F32, tag="xT", name="xT_init")
        nc.sync.dma_start(cur_x[:], d["xT0"][:])
        cur_h = st.tile([128, 128], F32, tag="hT", name="hT_init")   # 2*h, [h',(k,b)]
        nc.sync.dma_start(cur_h[:], d["h0T"][:])
        cur_c = st.tile([64, 256], F32, tag="cB", name="cB_init")    # [b, h]
        nc.sync.dma_start(cur_c[:], d["c0"][:])

        for t in range(L):
            # --- LSTM gates (flipped): gp[b, gdim] = x@W_ih.T + h@W_hh.T + bias
            gp = ps.tile([64, 1024], F32, tag="gp", name=f"gp{t}")
            srcs = [(cur_h, W_hhT, 0), (cur_h, W_hhT, 1),
                    (cur_x, W_ihT, 0), (cur_x, W_ihT, 1)]
            srcs = srcs[:2] + [(None, None, None)] + srcs[2:]
            for ki, (xv, Wv, kk) in enumerate(srcs):
                for c in range(2):
                    if xv is None:
                        nc.tensor.matmul(gp[:, c * 512:(c + 1) * 512],
                                         ones64[0:1, :],
                                         bias_row[0:1, c * 512:(c + 1) * 512],
                                         start=False, stop=False)
                    else:
                        nc.tensor.matmul(
                            gp[:, c * 512:(c + 1) * 512],
                            xv[:, kk * 64:(kk + 1) * 64],
                            Wv[:, kk * 1024 + c * 512:kk * 1024 + (c + 1) * 512],
                            start=(ki == 0), stop=(ki == 4))

            # sigmoid via tanh: sig(x) = 0.5*(1+tanh(x/2))
            TIF = wk.tile([64, 512], F32, tag="TIF", name=f"TIF{t}")
            nc.scalar.activation(TIF[:], gp[:, 0:512], AF.Tanh, scale=0.5)
            TG = wk.tile([64, 256], F32, tag="TG", name=f"TG{t}")
            nc.scalar.activation(TG[:], gp[:, 512:768], AF.Tanh)
            TO = wk.tile([64, 256], F32, tag="TO", name=f"TO{t}")
            nc.scalar.activation(TO[:], gp[:, 768:1024], AF.Tanh, scale=0.5)

            # c_new = 0.5*((1+tf)*c + (1+ti)*tg) ; h2 = (1+to)*tanh(c_new)
            A = wk.tile([64, 256], F32, tag="A", name=f"A{t}")
            nc.vector.scalar_tensor_tensor(A[:], in0=TIF[:, 256:512], scalar=1.0,
                                           in1=cur_c[:], op0=OP.add, op1=OP.mult)
            Bt = wk.tile([64, 256], F32, tag="Bt", name=f"Bt{t}")
            nc.vector.scalar_tensor_tensor(Bt[:], in0=TIF[:, 0:256], scalar=1.0,
                                           in1=TG[:], op0=OP.add, op1=OP.mult)
            Cp = wk.tile([64, 256], F32, tag="Cp", name=f"Cp{t}")
            nc.vector.tensor_tensor(Cp[:], A[:], Bt[:], op=OP.add)
            new_c = st.tile([64, 256], F32, tag="cB", name=f"cB{t}")
            nc.vector.tensor_scalar_mul(new_c[:], in0=Cp[:], scalar1=0.5)
            TC = wk.tile([64, 256], F32, tag="TC", name=f"TC{t}")
            nc.scalar.activation(TC[:], new_c[:], AF.Tanh)
            h2 = wk.tile([64, 256], F32, tag="h2", name=f"h2_{t}")
            nc.vector.scalar_tensor_tensor(h2[:], in0=TO[:], scalar=1.0,
                                           in1=TC[:], op0=OP.add, op1=OP.mult)
            cur_c = new_c

            # hT [h', (k,b)] via PE transposes (for next-step gates + qq1)
            htp = ps.tile([128, 128], F32, tag="htp", name=f"htp{t}")
            for k in range(2):
                nc.tensor.transpose(htp[:, k * 64:(k + 1) * 64],
                                    h2[:, k * 128:(k + 1) * 128], ident[0:64, 0:64])
            new_h = st.tile([128, 128], F32, tag="hT", name=f"hT{t}")
            nc.vector.tensor_copy(new_h[:], htp[:])
            cur_h = new_h

            # --- glimpse query qq1 (flipped): qq1f[b, o] = h2 @ (0.5*gl_Wq).T
            qq1f = ps.tile([64, 256], F32, tag="qq1f", name=f"qq1f{t}")
            for kk in range(2):
                nc.tensor.matmul(qq1f[:], cur_h[:, kk * 64:(kk + 1) * 64],
                                 glWqT[:, kk * 256:(kk + 1) * 256],
                                 start=(kk == 0), stop=(kk == 1))
            qq1p = wk.tile([128, 128], F32, tag="qq1p", name=f"qq1p{t}")
            nc.vector.tensor_copy(qq1p[0:64, :], qq1f[:, 0:128])
            nc.vector.tensor_copy(qq1p[64:128, :], qq1f[:, 128:256])

            # --- glimpse attention: u = sum_h v * tanh(E_gl + qq1)
            tpre = wk.tile([128, 4096], F32, tag="tpre", name=f"gtp{t}")
            for ch in [NCH - 1] + list(range(NCH - 1)):
                sl = slice(ch * CW, (ch + 1) * CW)
                eng = nc.gpsimd if ch == NCH - 1 else nc.vector
                eng.tensor_tensor(
                    tpre[:, sl].rearrange("p (l h) -> p l h", h=128),
                    E_gl[:, sl].rearrange("p (l h) -> p l h", h=128),
                    qq1p[:].unsqueeze(1).to_broadcast([128, LC, 128]), op=OP.add)
            u2g = ps.tile([128, 32], F32, tag="u2g", name=f"u2g{t}")
            for ch in range(NCH):
                sl = slice(ch * CW, (ch + 1) * CW)
                tact = wkc.tile([128, CW], F32, tag="tact", name=f"gta{t}_{ch}")
                nc.scalar.activation(tact[:], tpre[:, sl], AF.Tanh)
                tmul = wkc.tile([128, CW], F32, tag="tmul", name=f"gtm{t}_{ch}")
                nc.vector.tensor_tensor(
                    tmul[:].rearrange("p (l h) -> p l h", h=128),
                    tact[:].rearrange("p (l h) -> p l h", h=128),
                    glv[:].unsqueeze(1).to_broadcast([128, LC, 128]), op=OP.mult)
                nc.vector.tensor_reduce(u2g[:, ch * LC:(ch + 1) * LC],
                                        tmul[:].rearrange("p (l h) -> p l h", h=128),
                                        axis=AX.X, op=OP.add)
            uglo = wk.tile([64, 32], F32, tag="uglo", name=f"uglo{t}")
            nc.vector.tensor_copy(uglo[:], u2g[0:64, :])
            ug = wk.tile([128, 32], F32, tag="ug", name=f"ug{t}")
            nc.vector.tensor_tensor(ug[0:64, :], uglo[:], u2g[64:128, :], op=OP.add)
            nc.vector.tensor_tensor(ug[64:128, :], uglo[:], u2g[64:128, :], op=OP.add)
            ugm = wk.tile([128, 32], F32, tag="ugm", name=f"ugm{t}")
            nc.vector.tensor_tensor(ugm[:], ug[:], lnmask[:], op=OP.add)

            # softmax over l (rows duplicated)
            nmax = wk.tile([128, 1], F32, tag="nmax", name=f"nmax{t}")
            nc.vector.tensor_reduce(nmax[:], ugm[:], axis=AX.X, op=OP.max, negate=True)
            exg = wk.tile([128, 32], F32, tag="exg", name=f"exg{t}")
            ssum = wk.tile([128, 1], F32, tag="ssum", name=f"ssum{t}")
            nc.scalar.activation(exg[:], ugm[:], AF.Exp, bias=nmax[:, 0:1],
                                 accum_out=ssum[:, 0:1])
            rs = wk.tile([128, 1], F32, tag="rs", name=f"rs{t}")
            nc.vector.reciprocal(rs[:], ssum[:])
            p = wk.tile([128, 32], F32, tag="p", name=f"p{t}")
            nc.vector.tensor_scalar_mul(p[:], in0=exg[:], scalar1=rs[:, 0:1])

            # --- readout: qq2 = sum_l p * F  (F packed (h', l))
            qq2p = wk.tile([128, 128], F32, tag="qq2p", name=f"qq2p{t}")
            m2t = {}
            for ch in [NCH - 1] + list(range(NCH - 1)):
                sl = slice(ch * CW, (ch + 1) * CW)
                m2t[ch] = wkc.tile([128, CW], F32, tag="tmul", name=f"rm{t}_{ch}")
                eng = nc.gpsimd if ch == NCH - 1 else nc.vector
                eng.tensor_tensor(
                    m2t[ch][:].rearrange("p (h l) -> p h l", l=32),
                    F_p[:, sl].rearrange("p (h l) -> p h l", l=32),
                    p[:].unsqueeze(1).to_broadcast([128, HC, 32]), op=OP.mult)
            for ch in range(NCH):
                nc.vector.tensor_reduce(qq2p[:, ch * HC:(ch + 1) * HC],
                                        m2t[ch][:].rearrange("p (h l) -> p h l", l=32),
                                        axis=AX.X, op=OP.add)

            # --- pointer attention
            tpre2 = wk.tile([128, 4096], F32, tag="tpre", name=f"ptp{t}")
            for ch in [NCH - 1] + list(range(NCH - 1)):
                sl = slice(ch * CW, (ch + 1) * CW)
                eng = nc.gpsimd if ch == NCH - 1 else nc.vector
                eng.tensor_tensor(
                    tpre2[:, sl].rearrange("p (l h) -> p l h", h=128),
                    E_ptr[:, sl].rearrange("p (l h) -> p l h", h=128),
                    qq2p[:].unsqueeze(1).to_broadcast([128, LC, 128]), op=OP.add)
            u2b = ps.tile([128, 32], F32, tag="u2b", name=f"u2b{t}")
            for ch in range(NCH):
                sl = slice(ch * CW, (ch + 1) * CW)
                tact2 = wkc.tile([128, CW], F32, tag="tact", name=f"pta{t}_{ch}")
                nc.scalar.activation(tact2[:], tpre2[:, sl], AF.Tanh)
                tmul2 = wkc.tile([128, CW], F32, tag="tmul", name=f"ptm{t}_{ch}")
                nc.vector.tensor_tensor(
                    tmul2[:].rearrange("p (l h) -> p l h", h=128),
                    tact2[:].rearrange("p (l h) -> p l h", h=128),
                    ptrv[:].unsqueeze(1).to_broadcast([128, LC, 128]), op=OP.mult)
                nc.vector.tensor_reduce(u2b[:, ch * LC:(ch + 1) * LC],
                                        tmul2[:].rearrange("p (l h) -> p l h", h=128),
                                        axis=AX.X, op=OP.add)
            uplo = wk.tile([64, 32], F32, tag="uglo", name=f"uplo{t}")
            nc.vector.tensor_copy(uplo[:], u2b[0:64, :])
            up = wk.tile([128, 32], F32, tag="up", name=f"up{t}")
            nc.vector.tensor_tensor(up[0:64, :], uplo[:], u2b[64:128, :], op=OP.add)
            nc.vector.tensor_tensor(up[64:128, :], uplo[:], u2b[64:128, :], op=OP.add)

            tl = wk.tile([128, 32], F32, tag="tl", name=f"tl{t}")
            nc.scalar.activation(tl[:], up[:], AF.Tanh)
            lgm = wk.tile([128, 32], F32, tag="lgm", name=f"lgm{t}")
            nc.vector.scalar_tensor_tensor(lgm[:], in0=tl[:], scalar=C_TANH,
                                           in1=lnmask[:], op0=OP.mult, op1=OP.add)

            # argmax (first occurrence)
            m8 = wk.tile([128, 8], F32, tag="m8", name=f"m8_{t}")
            nc.vector.max(m8[:], lgm[:])
            mi8 = wk.tile([128, 8], U32, tag="mi8", name=f"mi8_{t}")
            nc.vector.max_index(mi8[:], m8[:], lgm[:])
            selsf1 = wk.tile([128, 1], F32, tag="selsf1", name=f"selsf1_{t}")
            nc.vector.tensor_copy(selsf1[:], mi8[:, 0:1])
            nc.vector.tensor_copy(sels_f[:, t:t + 1], selsf1[0:64, :])

            onehot = wk.tile([128, 32], I32, tag="onehot", name=f"onehot{t}")
            nc.vector.tensor_scalar(onehot[:], in0=iotaL[:], scalar1=selsf1[:, 0:1],
                                    scalar2=0.0, op0=OP.is_equal)

            # deferred log_softmax pieces: lgm -> logp_buf, -max -> nmax_buf,
            # sum(exp) -> s2_buf  (ln + assembly happen once after the loop)
            nc.vector.tensor_copy(logp_buf[:, t * 32:(t + 1) * 32], lgm[0:64, :])
            nmax2 = wk.tile([128, 1], F32, tag="nmax2", name=f"nmax2_{t}")
            nc.vector.tensor_reduce(nmax2[:], lgm[:], axis=AX.X, op=OP.max, negate=True)
            nc.vector.tensor_copy(nmax_buf[:, t:t + 1], nmax2[0:64, :])
            ex2 = wk.tile([64, 32], F32, tag="ex2", name=f"ex2_{t}")
            nc.scalar.activation(ex2[:], lgm[0:64, :], AF.Exp, bias=nmax2[0:64, 0:1],
                                 accum_out=s2_buf[:, t:t + 1])

            # mask update + mask_modify
            nc.vector.copy_predicated(lnmask[:], onehot[:], minf[:])
            rmx = wk.tile([128, 1], F32, tag="rmx", name=f"rmx{t}")
            nc.vector.tensor_reduce(rmx[:], lnmask[:], axis=AX.X, op=OP.max)
            cond = wk.tile([128, 1], I32, tag="cond", name=f"cond{t}")
            nc.vector.tensor_tensor(cond[:], rmx[:], minf[:, 0:1], op=OP.is_equal)
            nc.vector.copy_predicated(lnmask[:, 31:32], cond[:], zeros1[:])

            # gather next input: x_next[b,:] = emb[idx_b, b, :]
            if t + 1 < L:
                offs = wk.tile([64, 1], I32, tag="offs", name=f"offs{t}")
                nc.vector.tensor_tensor(offs[:], iotaB32[:], mi8[0:64, 0:1], op=OP.add)
                xg = wk.tile([64, 256], F32, tag="xg", name=f"xg{t}")
                nc.gpsimd.indirect_dma_start(
                    out=xg[:], out_offset=None, in_=d["embBL"][:],
                    in_offset=bass.IndirectOffsetOnAxis(ap=offs[:, 0:1], axis=0))
                xtps = ps.tile([128, 128], F32, tag="xtps", name=f"xtps{t}")
                for k in range(2):
                    nc.tensor.transpose(xtps[:, k * 64:(k + 1) * 64],
                                        xg[:, k * 128:(k + 1) * 128],
                                        ident[0:64, 0:64])
                new_x = st.tile([128, 128], F32, tag="xT", name=f"xT{t}")
                nc.vector.tensor_copy(new_x[:], xtps[:])
                cur_x = new_x

        # ---------- deferred log_softmax finalization + outputs ----------
        lns_all = pp.tile([64, 32], F32, tag="lns_all", name="lns_all")
        nc.scalar.activation(lns_all[:], s2_buf[:], AF.Ln)
        adj = pp.tile([64, 32], F32, tag="adj", name="adj")
        nc.vector.tensor_tensor(adj[:], nmax_buf[:], lns_all[:], op=OP.subtract)
        nc.vector.tensor_tensor(
            logp_fin[:].rearrange("p (t l) -> p t l", l=32),
            logp_buf[:].rearrange("p (t l) -> p t l", l=32),
            adj[:].unsqueeze(2).to_broadcast([64, 32, 32]), op=OP.add)

        sels_i = pp.tile([64, 32], I32, tag="sels_i", name="sels_i")
        nc.vector.tensor_copy(sels_i[:], sels_f[:])
        nc.sync.dma_start(logp_o[:], logp_fin[:])
        nc.sync.dma_start(sels_o[:], sels_i[:])


# ----------------------------------------------------------------------------
# host-side shard prep
# ----------------------------------------------------------------------------
def _halves(a):
    """[256, n] -> [128, 2n] with col-block k = rows 128k:128(k+1)."""
    return np.ascontiguousarray(np.concatenate([a[0:128], a[128:256]], axis=1),
                                dtype=np.float32)


def _mask_modify_np(m):
    out = m.copy()
    allt = out.all(axis=1)
    out[allt, -1] = False
    return out


def _prep_consts(inputs):
    """Core-independent tensors (weights, biases, constants)."""
    c = {}
    c["W_ihT"] = _halves(np.ascontiguousarray(inputs["W_ih"].T))
    c["W_hhT"] = _halves(np.ascontiguousarray(inputs["W_hh"].T * 0.5))   # h stored as 2h
    c["glWqT"] = _halves(np.ascontiguousarray(inputs["gl_Wq"].T * 0.5))  # h stored as 2h
    c["ptrWqT"] = _halves(np.ascontiguousarray(inputs["ptr_Wq"].T))
    c["glWrT"] = _halves(np.ascontiguousarray(inputs["gl_Wr"].T))
    c["glWr_raw"] = _halves(np.ascontiguousarray(inputs["gl_Wr"]))
    c["ptrWrT"] = _halves(np.ascontiguousarray(inputs["ptr_Wr"].T))
    c["bias_ih"] = np.ascontiguousarray(inputs["b_ih"].reshape(1, 1024), dtype=np.float32)
    c["bias_hh"] = np.ascontiguousarray(inputs["b_hh"].reshape(1, 1024), dtype=np.float32)
    for nm, key in (("glbq", "gl_bq"), ("glbr", "gl_br"),
                    ("ptrbq", "ptr_bq"), ("ptrbr", "ptr_br")):
        c[nm] = _halves(np.ascontiguousarray(np.asarray(inputs[key]).reshape(256, 1)))
    for nm, key in (("glv_rep", "gl_v"), ("ptrv_rep", "ptr_v")):
        v = np.asarray(inputs[key], dtype=np.float32)
        rep = np.empty((128, 128), dtype=np.float32)
        rep[0:64, :] = v[0:128][None, :]
        rep[64:128, :] = v[128:256][None, :]
        c[nm] = rep
    c["ones64"] = np.ones((1, 64), dtype=np.float32)
    c["ident"] = np.eye(128, dtype=np.float32)
    c["iotaL"] = np.tile(np.arange(32, dtype=np.float32), (128, 1))
    c["iotaB32"] = (np.arange(64, dtype=np.int32) * 32).reshape(64, 1)
    c["minf"] = np.full((128, 32), NEG, dtype=np.float32)
    c["zeros1"] = np.zeros((128, 1), dtype=np.float32)
    return c


def _prep_core(inputs, consts, core):
    b0, b1 = core * BL, (core + 1) * BL
    m = dict(consts)
    m["xT0"] = _halves(np.ascontiguousarray(np.asarray(inputs["decoder_input"])[b0:b1].T))
    m["h0T"] = _halves(np.ascontiguousarray(np.asarray(inputs["h0"])[b0:b1].T * 2.0))
    m["c0"] = np.ascontiguousarray(np.asarray(inputs["c0"])[b0:b1], dtype=np.float32)
    ctx = np.ascontiguousarray(np.asarray(inputs["context"])[:, b0:b1, :], dtype=np.float32)
    ctxT = ctx.transpose(2, 1, 0).reshape(256, BL * L)      # [h, (b,l)]
    m["ctxTd"] = _halves(ctxT)
    emb = np.ascontiguousarray(np.asarray(inputs["embedded_inputs"])[:, b0:b1, :],
                               dtype=np.float32)
    m["embBL"] = np.ascontiguousarray(emb.transpose(1, 0, 2).reshape(BL * L, 256))
    mask0 = _mask_modify_np(np.asarray(inputs["V_reach_mask"])[b0:b1].astype(bool))
    ln = np.where(mask0, NEG, np.float32(0.0)).astype(np.float32)
    m["lnmask0"] = np.concatenate([ln, ln], axis=0)          # duplicated rows
    return m


# ----------------------------------------------------------------------------
# entry point
# ----------------------------------------------------------------------------
def kernel(**inputs):
    global _PROG
    if _PROG is None:
        _PROG = _build()
    from concourse import bass_utils
    inputs = {k: np.asarray(v) for k, v in inputs.items()}
    consts = _prep_consts(inputs)
    in_maps = [_prep_core(inputs, consts, c) for c in range(NC)]
    trace = bool(int(os.environ.get("KERNEL_TRACE", "0")))
    tkw = {}
    if trace:
        tdir = os.environ.get("KERNEL_TRACE_DIR", "/root/problem/work/trace")
        import shutil
        shutil.rmtree(tdir, ignore_errors=True)
        os.makedirs(tdir, exist_ok=True)
        tkw["tmpdir"] = tdir
    res = bass_utils.run_bass_kernel_spmd(
        _PROG, in_maps, core_ids=list(range(NC)), trace=trace, **tkw)
    if trace and res.exec_time_ns is not None:
        kernel.last_exec_ns = res.exec_time_ns
        kernel.last_profile = res.profile_json
    logp = np.concatenate([res.results[c]["logp_o"].reshape(BL, L, L)
                           for c in range(NC)], axis=0)
    sels = np.concatenate([res.results[c]["sels_o"] for c in range(NC)],
                          axis=0).astype(np.int32)
    return logp, sels


# revision 16
# speedup vs baseline: 1.2066x; 1.2066x over previous
"""TRN2 Bass kernel for nn_DeepRoute (pointer-network greedy decoder).

Self-contained: hardcodes shapes, shards batch over 8 NeuronCores,
runs one SPMD Bass program per core, gathers full outputs.

Layouts (per core, b = 64 local batch rows):
  packed attention tiles: partitions r = hh*64 + b (hh = h-half), free:
    E_gl/E_ptr: (l, h') "lh";  F: (h', l) "hl"
  LSTM state: c [64(b), 256(h)]; h kept as h2 = 2*h in [64, 256] and
  transposed hT [128(h'), (k,b)]; weights host-pre-scaled to absorb the 2x
  (sigmoid computed as 0.5*(1+tanh(x/2)) -- only tanh/exp ACT tables used).
"""
import sys, os
sys.path.insert(0, "/opt/trn_rl_repo")
import numpy as np

L, B, E, H, NC = 32, 512, 256, 256, 8
BL = B // NC          # 64 batch rows per core
C_TANH = 10.0
NEG = np.float32(-np.inf)

_PROG = None  # cached compiled Bacc program


# ----------------------------------------------------------------------------
# program builder
# ----------------------------------------------------------------------------
def _build():
    import concourse.bass as bass
    import concourse.mybir as mybir
    import concourse.tile as tile
    from concourse import bacc

    F32, I32 = mybir.dt.float32, mybir.dt.int32

    nc = bacc.Bacc("TRN2", target_bir_lowering=False, debug=False,
                   enable_asserts=False)

    d = {}
    def din(name, shape, dt=F32):
        d[name] = nc.dram_tensor(name, shape, dt, kind="ExternalInput").ap()
        return d[name]

    # per-core inputs (host-packed layouts; see _prep_core)
    din("xT0", [128, 128]); din("h0T", [128, 128]); din("c0", [64, 256])
    din("ctxTd", [128, 4096])
    din("W_ihT", [128, 2048]); din("W_hhT", [128, 2048])
    din("glWqT", [128, 512]); din("ptrWqT", [128, 512])
    din("glWrT", [128, 512]); din("ptrWrT", [128, 512]); din("glWr_raw", [128, 512])
    din("bias_ih", [1, 1024]); din("bias_hh", [1, 1024])
    din("glbq", [128, 2]); din("glbr", [128, 2])
    din("ptrbq", [128, 2]); din("ptrbr", [128, 2])
    din("glv_rep", [128, 128]); din("ptrv_rep", [128, 128])
    din("ones64", [1, 64]); din("ident", [128, 128])
    din("iotaL", [128, 32]); din("iotaB32", [64, 1], I32)
    din("minf", [128, 32]); din("zeros1", [128, 1])
    din("lnmask0", [128, 32])
    din("embBL", [BL * L, 256])

    logp_o = nc.dram_tensor("logp_o", [BL, L * L], F32, kind="ExternalOutput").ap()
    sels_o = nc.dram_tensor("sels_o", [BL, L], I32, kind="ExternalOutput").ap()

    with tile.TileContext(nc) as tc:
        _emit(tc, nc, d, logp_o, sels_o, bass, mybir)
    nc.compile()
    return nc


def _emit(tc, nc, d, logp_o, sels_o, bass, mybir):
    F32, I32, U32 = mybir.dt.float32, mybir.dt.int32, mybir.dt.uint32
    AF, OP, AX = (mybir.ActivationFunctionType, mybir.AluOpType,
                  mybir.AxisListType)
    NCH = 4                      # chunks per big DVE op
    CW = 4096 // NCH             # chunk width (cols)
    LC = 32 // NCH               # l per chunk (lh layout)
    HC = 128 // NCH              # h' per chunk (hl layout)

    from contextlib import ExitStack
    with ExitStack() as ctx:
        pp = ctx.enter_context(tc.tile_pool(name="pp", bufs=1))
        st = ctx.enter_context(tc.tile_pool(name="st", bufs=2))

        # ---------- persistent SBUF ----------
        def load(pool, name, shape, dt=F32):
            t = pool.tile(shape, dt, tag=name, name=f"sb_{name}")
            nc.sync.dma_start(t[:], d[name][:])
            return t

        W_ihT = load(pp, "W_ihT", [128, 2048])
        W_hhT = load(pp, "W_hhT", [128, 2048])
        glWqT = load(pp, "glWqT", [128, 512])
        glv = load(pp, "glv_rep", [128, 128]); ptrv = load(pp, "ptrv_rep", [128, 128])
        ones64 = load(pp, "ones64", [1, 64]); ident = load(pp, "ident", [128, 128])
        iotaL = load(pp, "iotaL", [128, 32]); iotaB32 = load(pp, "iotaB32", [64, 1], I32)
        minf = load(pp, "minf", [128, 32]); zeros1 = load(pp, "zeros1", [128, 1])

        lnmask = pp.tile([128, 32], F32, tag="lnmask", name="lnmask")
        nc.sync.dma_start(lnmask[:], d["lnmask0"][:])

        E_gl = pp.tile([128, 4096], F32, tag="E_gl", name="E_gl")     # (l, h')
        E_ptr = pp.tile([128, 4096], F32, tag="E_ptr", name="E_ptr")  # (l, h')
        F_p = pp.tile([128, 4096], F32, tag="F_p", name="F_p")        # (h', l)
        logp_buf = pp.tile([64, 1024], F32, tag="logp_buf", name="logp_buf")
        logp_fin = pp.tile([64, 1024], F32, tag="logp_fin", name="logp_fin")
        nmax_buf = pp.tile([64, 32], F32, tag="nmax_buf", name="nmax_buf")
        s2_buf = pp.tile([64, 32], F32, tag="s2_buf", name="s2_buf")
        sels_f = pp.tile([64, 32], F32, tag="sels_f", name="sels_f")
        bias_row = pp.tile([1, 1024], F32, tag="bias_row", name="bias_row")

        # ---------- precompute: E_gl, E_ptr, F (one-time) ----------
        with tc.tile_pool(name="pre", bufs=1) as pre, \
             tc.tile_pool(name="preps", bufs=1, space="PSUM") as preps:
            ctxT = load(pre, "ctxTd", [128, 4096])
            glWrT = load(pre, "glWrT", [128, 512])
            ptrWrT = load(pre, "ptrWrT", [128, 512])
            ptrWqT = load(pre, "ptrWqT", [128, 512])
            glWr_raw = load(pre, "glWr_raw", [128, 512])
            b_ih = load(pre, "bias_ih", [1, 1024])
            b_hh = load(pre, "bias_hh", [1, 1024])
            glbq = load(pre, "glbq", [128, 2]); glbr = load(pre, "glbr", [128, 2])
            ptrbq = load(pre, "ptrbq", [128, 2]); ptrbr = load(pre, "ptrbr", [128, 2])

            nc.vector.tensor_tensor(bias_row[:], b_ih[:], b_hh[:], op=OP.add)
            bias_glE = pre.tile([128, 2], F32, tag="bias_glE", name="bias_glE")
            nc.vector.tensor_tensor(bias_glE[:], glbr[:], glbq[:], op=OP.add)
            bias_ptrE = pre.tile([128, 2], F32, tag="bias_ptrE", name="bias_ptrE")
            nc.vector.tensor_tensor(bias_ptrE[:], ptrbr[:], ptrbq[:], op=OP.add)

            # WfT[e, o] = sum_h gl_Wr[h, e] * ptr_Wq.T[h, o]  (for F = ctx@Wf.T)
            WfT = pre.tile([128, 512], F32, tag="WfT", name="WfT")
            for m in range(2):     # e-half
                wps = preps.tile([128, 256], F32, tag="wps", name=f"wps{m}")
                for k in range(2):  # h-half
                    nc.tensor.matmul(
                        wps[:], glWr_raw[:, k * 256 + m * 128:k * 256 + m * 128 + 128],
                        ptrWqT[:, k * 256:(k + 1) * 256],
                        start=(k == 0), stop=(k == 1))
                nc.vector.tensor_copy(WfT[:, m * 256:(m + 1) * 256], wps[:])

            # bias for F: bf = ptr_Wq @ gl_br
            bfp = preps.tile([128, 2], F32, tag="bfp", name="bfp")
            for hh in range(2):
                for k in range(2):
                    nc.tensor.matmul(bfp[:, hh:hh + 1],
                                     ptrWqT[:, k * 256 + hh * 128:k * 256 + hh * 128 + 128],
                                     glbr[:, k:k + 1],
                                     start=(k == 0), stop=(k == 1))
            bias_F = pre.tile([128, 2], F32, tag="bias_F", name="bias_F")
            nc.vector.tensor_copy(bias_F[:], bfp[:])

            # stream each big tensor: mm chunk -> stage (+bias) -> transpose-repack
            def emit_packed(lhsT, biasE, dst, hl, name):
                for hh in range(2):          # output h-half
                    eps = preps.tile([128, 2048], F32, tag="eps", name=f"eps_{name}{hh}")
                    for c in range(4):
                        for k in range(2):
                            nc.tensor.matmul(
                                eps[:, c * 512:(c + 1) * 512],
                                lhsT[:, k * 256 + hh * 128:k * 256 + hh * 128 + 128],
                                ctxT[:, k * 2048 + c * 512:k * 2048 + (c + 1) * 512],
                                start=(k == 0), stop=(k == 1))
                    stage = pre.tile([128, 2048], F32, tag="stage", name=f"stg_{name}{hh}")
                    nc.vector.tensor_scalar(stage[:], in0=eps[:],
                                            scalar1=biasE[:, hh:hh + 1],
                                            scalar2=0.0, op0=OP.add)
                    st3 = stage[:].rearrange("p (b l) -> p b l", l=32)
                    for l in range(32):
                        tp = preps.tile([64, 128], F32, tag="tp", name=f"tp_{name}{hh}_{l}")
                        nc.tensor.transpose(tp[:], st3[:, :, l], ident[:])
                        if hl:
                            dstv = dst[hh * 64:(hh + 1) * 64, :] \
                                .rearrange("p (h l) -> p h l", l=32)[:, :, l]
                        else:
                            dstv = dst[hh * 64:(hh + 1) * 64, l * 128:(l + 1) * 128]
                        nc.vector.tensor_copy(dstv, tp[:])

            emit_packed(glWrT, bias_glE, E_gl, False, "gl")
            emit_packed(ptrWrT, bias_ptrE, E_ptr, False, "ptr")
            emit_packed(WfT, bias_F, F_p, True, "F")

        # ---------- working pools (opened after precompute frees space) ----------
        wk = ctx.enter_context(tc.tile_pool(name="wk", bufs=2))
        wkc = ctx.enter_context(tc.tile_pool(name="wkc", bufs=4))
        ps = ctx.enter_context(tc.tile_pool(name="ps", bufs=1, space="PSUM"))

        cur_x = st.tile([128, 128], F32, tag="xT", name="xT_init")
        nc.sync.dma_start(cur_x[:], d["xT0"][:])
        cur_h = st.tile([128, 128], F32, tag="hT", name="hT_init")   # 2*h, [h',(k,b)]
        nc.sync.dma_start(cur_h[:], d["h0T"][:])
        cur_c = st.tile([64, 256], F32, tag="cB", name="cB_init")    # [b, h]
        nc.sync.dma_start(cur_c[:], d["c0"][:])

        for t in range(L):
            # --- LSTM gates (flipped): gp[b, gdim] = x@W_ih.T + h@W_hh.T + bias
            gp = ps.tile([64, 1024], F32, tag="gp", name=f"gp{t}")
            srcs = [(cur_h, W_hhT, 0), (cur_h, W_hhT, 1),
                    (cur_x, W_ihT, 0), (cur_x, W_ihT, 1)]
            srcs = srcs[:2] + [(None, None, None)] + srcs[2:]
            for ki, (xv, Wv, kk) in enumerate(srcs):
                for c in range(2):
                    if xv is None:
                        nc.tensor.matmul(gp[:, c * 512:(c + 1) * 512],
                                         ones64[0:1, :],
                                         bias_row[0:1, c * 512:(c + 1) * 512],
                                         start=False, stop=False)
                    else:
                        nc.tensor.matmul(
                            gp[:, c * 512:(c + 1) * 512],
                            xv[:, kk * 64:(kk + 1) * 64],
                            Wv[:, kk * 1024 + c * 512:kk * 1024 + (c + 1) * 512],
                            start=(ki == 0), stop=(ki == 4))

            # sigmoid via tanh: sig(x) = 0.5*(1+tanh(x/2))
            TIF = wk.tile([64, 512], F32, tag="TIF", name=f"TIF{t}")
            nc.scalar.activation(TIF[:], gp[:, 0:512], AF.Tanh, scale=0.5)
            TG = wk.tile([64, 256], F32, tag="TG", name=f"TG{t}")
            nc.scalar.activation(TG[:], gp[:, 512:768], AF.Tanh)
            TO = wk.tile([64, 256], F32, tag="TO", name=f"TO{t}")
            nc.scalar.activation(TO[:], gp[:, 768:1024], AF.Tanh, scale=0.5)

            # c_new = 0.5*((1+tf)*c + (1+ti)*tg) ; h2 = (1+to)*tanh(c_new)
            A = wk.tile([64, 256], F32, tag="A", name=f"A{t}")
            nc.vector.scalar_tensor_tensor(A[:], in0=TIF[:, 256:512], scalar=1.0,
                                           in1=cur_c[:], op0=OP.add, op1=OP.mult)
            Bt = wk.tile([64, 256], F32, tag="Bt", name=f"Bt{t}")
            nc.vector.scalar_tensor_tensor(Bt[:], in0=TIF[:, 0:256], scalar=1.0,
                                           in1=TG[:], op0=OP.add, op1=OP.mult)
            Cp = wk.tile([64, 256], F32, tag="Cp", name=f"Cp{t}")
            nc.vector.tensor_tensor(Cp[:], A[:], Bt[:], op=OP.add)
            new_c = st.tile([64, 256], F32, tag="cB", name=f"cB{t}")
            nc.vector.tensor_scalar_mul(new_c[:], in0=Cp[:], scalar1=0.5)
            TC = wk.tile([64, 256], F32, tag="TC", name=f"TC{t}")
            nc.scalar.activation(TC[:], new_c[:], AF.Tanh)
            h2 = wk.tile([64, 256], F32, tag="h2", name=f"h2_{t}")
            nc.vector.scalar_tensor_tensor(h2[:], in0=TO[:], scalar=1.0,
                                           in1=TC[:], op0=OP.add, op1=OP.mult)
            cur_c = new_c

            # hT [h', (k,b)] via PE transposes (for next-step gates + qq1)
            htp = ps.tile([128, 128], F32, tag="htp", name=f"htp{t}")
            for k in range(2):
                nc.tensor.transpose(htp[:, k * 64:(k + 1) * 64],
                                    h2[:, k * 128:(k + 1) * 128], ident[0:64, 0:64])
            new_h = st.tile([128, 128], F32, tag="hT", name=f"hT{t}")
            nc.vector.tensor_copy(new_h[:], htp[:])
            cur_h = new_h

            # --- glimpse query qq1 (flipped): qq1f[b, o] = h2 @ (0.5*gl_Wq).T
            qq1f = ps.tile([64, 256], F32, tag="qq1f", name=f"qq1f{t}")
            for kk in range(2):
                nc.tensor.matmul(qq1f[:], cur_h[:, kk * 64:(kk + 1) * 64],
                                 glWqT[:, kk * 256:(kk + 1) * 256],
                                 start=(kk == 0), stop=(kk == 1))
            qq1p = wk.tile([128, 128], F32, tag="qq1p", name=f"qq1p{t}")
            nc.vector.tensor_copy(qq1p[0:64, :], qq1f[:, 0:128])
            nc.vector.tensor_copy(qq1p[64:128, :], qq1f[:, 128:256])

            # --- glimpse attention: u = sum_h v * tanh(E_gl + qq1)
            tpre = wk.tile([128, 4096], F32, tag="tpre", name=f"gtp{t}")
            for ch in range(NCH):
                sl = slice(ch * CW, (ch + 1) * CW)
                eng = nc.gpsimd if ch == 0 else nc.vector
                eng.tensor_tensor(
                    tpre[:, sl].rearrange("p (l h) -> p l h", h=128),
                    E_gl[:, sl].rearrange("p (l h) -> p l h", h=128),
                    qq1p[:].unsqueeze(1).to_broadcast([128, LC, 128]), op=OP.add)
            u2g = ps.tile([128, 32], F32, tag="u2g", name=f"u2g{t}")
            for ch in range(NCH):
                sl = slice(ch * CW, (ch + 1) * CW)
                tact = wkc.tile([128, CW], F32, tag="tact", name=f"gta{t}_{ch}")
                nc.scalar.activation(tact[:], tpre[:, sl], AF.Tanh)
                tmul = wkc.tile([128, CW], F32, tag="tmul", name=f"gtm{t}_{ch}")
                nc.vector.tensor_tensor(
                    tmul[:].rearrange("p (l h) -> p l h", h=128),
                    tact[:].rearrange("p (l h) -> p l h", h=128),
                    glv[:].unsqueeze(1).to_broadcast([128, LC, 128]), op=OP.mult)
                nc.vector.tensor_reduce(u2g[:, ch * LC:(ch + 1) * LC],
                                        tmul[:].rearrange("p (l h) -> p l h", h=128),
                                        axis=AX.X, op=OP.add)
            uglo = wk.tile([64, 32], F32, tag="uglo", name=f"uglo{t}")
            nc.vector.tensor_copy(uglo[:], u2g[0:64, :])
            ug = wk.tile([128, 32], F32, tag="ug", name=f"ug{t}")
            nc.vector.tensor_tensor(ug[0:64, :], uglo[:], u2g[64:128, :], op=OP.add)
            nc.vector.tensor_tensor(ug[64:128, :], uglo[:], u2g[64:128, :], op=OP.add)
            ugm = wk.tile([128, 32], F32, tag="ugm", name=f"ugm{t}")
            nc.vector.tensor_tensor(ugm[:], ug[:], lnmask[:], op=OP.add)

            # softmax over l (rows duplicated)
            nmax = wk.tile([128, 1], F32, tag="nmax", name=f"nmax{t}")
            nc.vector.tensor_reduce(nmax[:], ugm[:], axis=AX.X, op=OP.max, negate=True)
            exg = wk.tile([128, 32], F32, tag="exg", name=f"exg{t}")
            ssum = wk.tile([128, 1], F32, tag="ssum", name=f"ssum{t}")
            nc.scalar.activation(exg[:], ugm[:], AF.Exp, bias=nmax[:, 0:1],
                                 accum_out=ssum[:, 0:1])
            rs = wk.tile([128, 1], F32, tag="rs", name=f"rs{t}")
            nc.vector.reciprocal(rs[:], ssum[:])
            p = wk.tile([128, 32], F32, tag="p", name=f"p{t}")
            nc.vector.tensor_scalar_mul(p[:], in0=exg[:], scalar1=rs[:, 0:1])

            # --- readout: qq2 = sum_l p * F  (F packed (h', l))
            qq2p = wk.tile([128, 128], F32, tag="qq2p", name=f"qq2p{t}")
            for ch in range(NCH):
                sl = slice(ch * CW, (ch + 1) * CW)
                m2 = wkc.tile([128, CW], F32, tag="tmul", name=f"rm{t}_{ch}")
                nc.vector.tensor_tensor(
                    m2[:].rearrange("p (h l) -> p h l", l=32),
                    F_p[:, sl].rearrange("p (h l) -> p h l", l=32),
                    p[:].unsqueeze(1).to_broadcast([128, HC, 32]), op=OP.mult)
                nc.vector.tensor_reduce(qq2p[:, ch * HC:(ch + 1) * HC],
                                        m2[:].rearrange("p (h l) -> p h l", l=32),
                                        axis=AX.X, op=OP.add)

            # --- pointer attention
            tpre2 = wk.tile([128, 4096], F32, tag="tpre", name=f"ptp{t}")
            for ch in range(NCH):
                sl = slice(ch * CW, (ch + 1) * CW)
                eng = nc.gpsimd if ch == 0 else nc.vector
                eng.tensor_tensor(
                    tpre2[:, sl].rearrange("p (l h) -> p l h", h=128),
                    E_ptr[:, sl].rearrange("p (l h) -> p l h", h=128),
                    qq2p[:].unsqueeze(1).to_broadcast([128, LC, 128]), op=OP.add)
            u2b = ps.tile([128, 32], F32, tag="u2b", name=f"u2b{t}")
            for ch in range(NCH):
                sl = slice(ch * CW, (ch + 1) * CW)
                tact2 = wkc.tile([128, CW], F32, tag="tact", name=f"pta{t}_{ch}")
                nc.scalar.activation(tact2[:], tpre2[:, sl], AF.Tanh)
                tmul2 = wkc.tile([128, CW], F32, tag="tmul", name=f"ptm{t}_{ch}")
                nc.vector.tensor_tensor(
                    tmul2[:].rearrange("p (l h) -> p l h", h=128),
                    tact2[:].rearrange("p (l h) -> p l h", h=128),
                    ptrv[:].unsqueeze(1).to_broadcast([128, LC, 128]), op=OP.mult)
                nc.vector.tensor_reduce(u2b[:, ch * LC:(ch + 1) * LC],
                                        tmul2[:].rearrange("p (l h) -> p l h", h=128),
                                        axis=AX.X, op=OP.add)
            uplo = wk.tile([64, 32], F32, tag="uglo", name=f"uplo{t}")
            nc.vector.tensor_copy(uplo[:], u2b[0:64, :])
            up = wk.tile([128, 32], F32, tag="up", name=f"up{t}")
            nc.vector.tensor_tensor(up[0:64, :], uplo[:], u2b[64:128, :], op=OP.add)
            nc.vector.tensor_tensor(up[64:128, :], uplo[:], u2b[64:128, :], op=OP.add)

            tl = wk.tile([128, 32], F32, tag="tl", name=f"tl{t}")
            nc.scalar.activation(tl[:], up[:], AF.Tanh)
            lgm = wk.tile([128, 32], F32, tag="lgm", name=f"lgm{t}")
            nc.vector.scalar_tensor_tensor(lgm[:], in0=tl[:], scalar=C_TANH,
                                           in1=lnmask[:], op0=OP.mult, op1=OP.add)

            # argmax (first occurrence)
            m8 = wk.tile([128, 8], F32, tag="m8", name=f"m8_{t}")
            nc.vector.max(m8[:], lgm[:])
            mi8 = wk.tile([128, 8], U32, tag="mi8", name=f"mi8_{t}")
            nc.vector.max_index(mi8[:], m8[:], lgm[:])
            selsf1 = wk.tile([128, 1], F32, tag="selsf1", name=f"selsf1_{t}")
            nc.vector.tensor_copy(selsf1[:], mi8[:, 0:1])
            nc.vector.tensor_copy(sels_f[:, t:t + 1], selsf1[0:64, :])

            onehot = wk.tile([128, 32], I32, tag="onehot", name=f"onehot{t}")
            nc.vector.tensor_scalar(onehot[:], in0=iotaL[:], scalar1=selsf1[:, 0:1],
                                    scalar2=0.0, op0=OP.is_equal)

            # deferred log_softmax pieces: lgm -> logp_buf, -max -> nmax_buf,
            # sum(exp) -> s2_buf  (ln + assembly happen once after the loop)
            nc.vector.tensor_copy(logp_buf[:, t * 32:(t + 1) * 32], lgm[0:64, :])
            nmax2 = wk.tile([128, 1], F32, tag="nmax2", name=f"nmax2_{t}")
            nc.vector.tensor_reduce(nmax2[:], lgm[:], axis=AX.X, op=OP.max, negate=True)
            nc.vector.tensor_copy(nmax_buf[:, t:t + 1], nmax2[0:64, :])
            ex2 = wk.tile([64, 32], F32, tag="ex2", name=f"ex2_{t}")
            nc.scalar.activation(ex2[:], lgm[0:64, :], AF.Exp, bias=nmax2[0:64, 0:1],
                                 accum_out=s2_buf[:, t:t + 1])

            # mask update + mask_modify
            nc.vector.copy_predicated(lnmask[:], onehot[:], minf[:])
            rmx = wk.tile([128, 1], F32, tag="rmx", name=f"rmx{t}")
            nc.vector.tensor_reduce(rmx[:], lnmask[:], axis=AX.X, op=OP.max)
            cond = wk.tile([128, 1], I32, tag="cond", name=f"cond{t}")
            nc.vector.tensor_tensor(cond[:], rmx[:], minf[:, 0:1], op=OP.is_equal)
            nc.vector.copy_predicated(lnmask[:, 31:32], cond[:], zeros1[:])

            # gather next input: x_next[b,:] = emb[idx_b, b, :]
            if t + 1 < L:
                offs = wk.tile([64, 1], I32, tag="offs", name=f"offs{t}")
                nc.vector.tensor_tensor(offs[:], iotaB32[:], mi8[0:64, 0:1], op=OP.add)
                xg = wk.tile([64, 256], F32, tag="xg", name=f"xg{t}")
                nc.gpsimd.indirect_dma_start(
                    out=xg[:], out_offset=None, in_=d["embBL"][:],
                    in_offset=bass.IndirectOffsetOnAxis(ap=offs[:, 0:1], axis=0))
                xtps = ps.tile([128, 128], F32, tag="xtps", name=f"xtps{t}")
                for k in range(2):
                    nc.tensor.transpose(xtps[:, k * 64:(k + 1) * 64],
                                        xg[:, k * 128:(k + 1) * 128],
                                        ident[0:64, 0:64])
                new_x = st.tile([128, 128], F32, tag="xT", name=f"xT{t}")
                nc.vector.tensor_copy(new_x[:], xtps[:])
                cur_x = new_x

        # ---------- deferred log_softmax finalization + outputs ----------
        lns_all = pp.tile([64, 32], F32, tag="lns_all", name="lns_all")
        nc.scalar.activation(lns_all[:], s2_buf[:], AF.Ln)
        adj = pp.tile([64, 32], F32, tag="adj", name="adj")
        nc.vector.tensor_tensor(adj[:], nmax_buf[:], lns_all[:], op=OP.subtract)
        nc.vector.tensor_tensor(
            logp_fin[:].rearrange("p (t l) -> p t l", l=32),
            logp_buf[:].rearrange("p (t l) -> p t l", l=32),
            adj[:].unsqueeze(2).to_broadcast([64, 32, 32]), op=OP.add)

        sels_i = pp.tile([64, 32], I32, tag="sels_i", name="sels_i")
        nc.vector.tensor_copy(sels_i[:], sels_f[:])
        nc.sync.dma_start(logp_o[:], logp_fin[:])
        nc.sync.dma_start(sels_o[:], sels_i[:])


# ----------------------------------------------------------------------------
# host-side shard prep
# ----------------------------------------------------------------------------
def _halves(a):
    """[256, n] -> [128, 2n] with col-block k = rows 128k:128(k+1)."""
    return np.ascontiguousarray(np.concatenate([a[0:128], a[128:256]], axis=1),
                                dtype=np.float32)


def _mask_modify_np(m):
    out = m.copy()
    allt = out.all(axis=1)
    out[allt, -1] = False
    return out


def _prep_consts(inputs):
    """Core-independent tensors (weights, biases, constants)."""
    c = {}
    c["W_ihT"] = _halves(np.ascontiguousarray(inputs["W_ih"].T))
    c["W_hhT"] = _halves(np.ascontiguousarray(inputs["W_hh"].T * 0.5))   # h stored as 2h
    c["glWqT"] = _halves(np.ascontiguousarray(inputs["gl_Wq"].T * 0.5))  # h stored as 2h
    c["ptrWqT"] = _halves(np.ascontiguousarray(inputs["ptr_Wq"].T))
    c["glWrT"] = _halves(np.ascontiguousarray(inputs["gl_Wr"].T))
    c["glWr_raw"] = _halves(np.ascontiguousarray(inputs["gl_Wr"]))
    c["ptrWrT"] = _halves(np.ascontiguousarray(inputs["ptr_Wr"].T))
    c["bias_ih"] = np.ascontiguousarray(inputs["b_ih"].reshape(1, 1024), dtype=np.float32)
    c["bias_hh"] = np.ascontiguousarray(inputs["b_hh"].reshape(1, 1024), dtype=np.float32)
    for nm, key in (("glbq", "gl_bq"), ("glbr", "gl_br"),
                    ("ptrbq", "ptr_bq"), ("ptrbr", "ptr_br")):
        c[nm] = _halves(np.ascontiguousarray(np.asarray(inputs[key]).reshape(256, 1)))
    for nm, key in (("glv_rep", "gl_v"), ("ptrv_rep", "ptr_v")):
        v = np.asarray(inputs[key], dtype=np.float32)
        rep = np.empty((128, 128), dtype=np.float32)
        rep[0:64, :] = v[0:128][None, :]
        rep[64:128, :] = v[128:256][None, :]
        c[nm] = rep
    c["ones64"] = np.ones((1, 64), dtype=np.float32)
    c["ident"] = np.eye(128, dtype=np.float32)
    c["iotaL"] = np.tile(np.arange(32, dtype=np.float32), (128, 1))
    c["iotaB32"] = (np.arange(64, dtype=np.int32) * 32).reshape(64, 1)
    c["minf"] = np.full((128, 32), NEG, dtype=np.float32)
    c["zeros1"] = np.zeros((128, 1), dtype=np.float32)
    return c


def _prep_core(inputs, consts, core):
    b0, b1 = core * BL, (core + 1) * BL
    m = dict(consts)
    m["xT0"] = _halves(np.ascontiguousarray(np.asarray(inputs["decoder_input"])[b0:b1].T))
    m["h0T"] = _halves(np.ascontiguousarray(np.asarray(inputs["h0"])[b0:b1].T * 2.0))
    m["c0"] = np.ascontiguousarray(np.asarray(inputs["c0"])[b0:b1], dtype=np.float32)
    ctx = np.ascontiguousarray(np.asarray(inputs["context"])[:, b0:b1, :], dtype=np.float32)
    ctxT = ctx.transpose(2, 1, 0).reshape(256, BL * L)      # [h, (b,l)]
    m["ctxTd"] = _halves(ctxT)
    emb = np.ascontiguousarray(np.asarray(inputs["embedded_inputs"])[:, b0:b1, :],
                               dtype=np.float32)
    m["embBL"] = np.ascontiguousarray(emb.transpose(1, 0, 2).reshape(BL * L, 256))
    mask0 = _mask_modify_np(np.asarray(inputs["V_reach_mask"])[b0:b1].astype(bool))
    ln = np.where(mask0, NEG, np.float32(0.0)).astype(np.float32)
    m["lnmask0"] = np.concatenate([ln, ln], axis=0)          # duplicated rows
    return m


# ----------------------------------------------------------------------------
# entry point
# ----------------------------------------------------------------------------
def kernel(**inputs):
    global _PROG
    if _PROG is None:
        _PROG = _build()
    from concourse import bass_utils
    inputs = {k: np.asarray(v) for k, v in inputs.items()}
    consts = _prep_consts(inputs)
    in_maps = [_prep_core(inputs, consts, c) for c in range(NC)]
    trace = bool(int(os.environ.get("KERNEL_TRACE", "0")))
    tkw = {}
    if trace:
        tdir = os.environ.get("KERNEL_TRACE_DIR", "/root/problem/work/trace")
        import shutil
        shutil.rmtree(tdir, ignore_errors=True)
        os.makedirs(tdir, exist_ok=True)
        tkw["tmpdir"] = tdir
    res = bass_utils.run_bass_kernel_spmd(
        _PROG, in_maps, core_ids=list(range(NC)), trace=trace, **tkw)
    if trace and res.exec_time_ns is not None:
        kernel.last_exec_ns = res.exec_time_ns
        kernel.last_profile = res.profile_json
    logp = np.concatenate([res.results[c]["logp_o"].reshape(BL, L, L)
                           for c in range(NC)], axis=0)
    sels = np.concatenate([res.results[c]["sels_o"] for c in range(NC)],
                          axis=0).astype(np.int32)
    return logp, sels


# revision 17
# speedup vs baseline: 1.2431x; 1.0302x over previous
"""TRN2 Bass kernel for nn_DeepRoute (pointer-network greedy decoder).

Self-contained: hardcodes shapes, shards batch over 8 NeuronCores,
runs one SPMD Bass program per core, gathers full outputs.

Layouts (per core, b = 64 local batch rows):
  packed attention tiles: partitions r = hh*64 + b (hh = h-half), free:
    E_gl/E_ptr: (l, h') "lh";  F: (h', l) "hl"
  LSTM state: c [64(b), 256(h)]; h kept as h2 = 2*h in [64, 256] and
  transposed hT [128(h'), (k,b)]; weights host-pre-scaled to absorb the 2x
  (sigmoid computed as 0.5*(1+tanh(x/2)) -- only tanh/exp ACT tables used).
"""
import sys, os
sys.path.insert(0, "/opt/trn_rl_repo")
import numpy as np

L, B, E, H, NC = 32, 512, 256, 256, 8
BL = B // NC          # 64 batch rows per core
C_TANH = 10.0
NEG = np.float32(-np.inf)

_PROG = None  # cached compiled Bacc program


# ----------------------------------------------------------------------------
# program builder
# ----------------------------------------------------------------------------
def _build():
    import concourse.bass as bass
    import concourse.mybir as mybir
    import concourse.tile as tile
    from concourse import bacc

    F32, I32 = mybir.dt.float32, mybir.dt.int32

    nc = bacc.Bacc("TRN2", target_bir_lowering=False, debug=False,
                   enable_asserts=False)

    d = {}
    def din(name, shape, dt=F32):
        d[name] = nc.dram_tensor(name, shape, dt, kind="ExternalInput").ap()
        return d[name]

    # per-core inputs (host-packed layouts; see _prep_core)
    din("xT0", [128, 128]); din("h0T", [128, 128]); din("c0", [64, 256])
    din("ctxTd", [128, 4096])
    din("W_ihT", [128, 2048]); din("W_hhT", [128, 2048])
    din("glWqT", [128, 512]); din("ptrWqT", [128, 512])
    din("glWrT", [128, 512]); din("ptrWrT", [128, 512]); din("glWr_raw", [128, 512])
    din("bias_ih", [1, 1024]); din("bias_hh", [1, 1024])
    din("glbq", [128, 2]); din("glbr", [128, 2])
    din("ptrbq", [128, 2]); din("ptrbr", [128, 2])
    din("glv_rep", [128, 128]); din("ptrv_rep", [128, 128])
    din("ones64", [1, 64]); din("ident", [128, 128])
    din("iotaL", [128, 32]); din("iotaB32", [64, 1], I32)
    din("minf", [128, 32]); din("zeros1", [128, 1])
    din("lnmask0", [128, 32])
    din("embBL", [BL * L, 256])

    logp_o = nc.dram_tensor("logp_o", [BL, L * L], F32, kind="ExternalOutput").ap()
    sels_o = nc.dram_tensor("sels_o", [BL, L], I32, kind="ExternalOutput").ap()

    with tile.TileContext(nc) as tc:
        _emit(tc, nc, d, logp_o, sels_o, bass, mybir)
    nc.compile()
    return nc


def _emit(tc, nc, d, logp_o, sels_o, bass, mybir):
    F32, I32, U32 = mybir.dt.float32, mybir.dt.int32, mybir.dt.uint32
    AF, OP, AX = (mybir.ActivationFunctionType, mybir.AluOpType,
                  mybir.AxisListType)
    NCH = 4                      # chunks per big DVE op
    CW = 4096 // NCH             # chunk width (cols)
    LC = 32 // NCH               # l per chunk (lh layout)
    HC = 128 // NCH              # h' per chunk (hl layout)

    from contextlib import ExitStack
    with ExitStack() as ctx:
        pp = ctx.enter_context(tc.tile_pool(name="pp", bufs=1))
        st = ctx.enter_context(tc.tile_pool(name="st", bufs=2))

        # ---------- persistent SBUF ----------
        def load(pool, name, shape, dt=F32):
            t = pool.tile(shape, dt, tag=name, name=f"sb_{name}")
            nc.sync.dma_start(t[:], d[name][:])
            return t

        W_ihT = load(pp, "W_ihT", [128, 2048])
        W_hhT = load(pp, "W_hhT", [128, 2048])
        glWqT = load(pp, "glWqT", [128, 512])
        glv = load(pp, "glv_rep", [128, 128]); ptrv = load(pp, "ptrv_rep", [128, 128])
        ones64 = load(pp, "ones64", [1, 64]); ident = load(pp, "ident", [128, 128])
        iotaL = load(pp, "iotaL", [128, 32]); iotaB32 = load(pp, "iotaB32", [64, 1], I32)
        minf = load(pp, "minf", [128, 32]); zeros1 = load(pp, "zeros1", [128, 1])

        lnmask = pp.tile([128, 32], F32, tag="lnmask", name="lnmask")
        nc.sync.dma_start(lnmask[:], d["lnmask0"][:])

        E_gl = pp.tile([128, 4096], F32, tag="E_gl", name="E_gl")     # (l, h')
        E_ptr = pp.tile([128, 4096], F32, tag="E_ptr", name="E_ptr")  # (l, h')
        F_p = pp.tile([128, 4096], F32, tag="F_p", name="F_p")        # (h', l)
        logp_buf = pp.tile([64, 1024], F32, tag="logp_buf", name="logp_buf")
        logp_fin = pp.tile([64, 1024], F32, tag="logp_fin", name="logp_fin")
        nmax_buf = pp.tile([64, 32], F32, tag="nmax_buf", name="nmax_buf")
        s2_buf = pp.tile([64, 32], F32, tag="s2_buf", name="s2_buf")
        sels_f = pp.tile([64, 32], F32, tag="sels_f", name="sels_f")
        bias_row = pp.tile([1, 1024], F32, tag="bias_row", name="bias_row")

        # ---------- precompute: E_gl, E_ptr, F (one-time) ----------
        with tc.tile_pool(name="pre", bufs=1) as pre, \
             tc.tile_pool(name="preps", bufs=1, space="PSUM") as preps:
            ctxT = load(pre, "ctxTd", [128, 4096])
            glWrT = load(pre, "glWrT", [128, 512])
            ptrWrT = load(pre, "ptrWrT", [128, 512])
            ptrWqT = load(pre, "ptrWqT", [128, 512])
            glWr_raw = load(pre, "glWr_raw", [128, 512])
            b_ih = load(pre, "bias_ih", [1, 1024])
            b_hh = load(pre, "bias_hh", [1, 1024])
            glbq = load(pre, "glbq", [128, 2]); glbr = load(pre, "glbr", [128, 2])
            ptrbq = load(pre, "ptrbq", [128, 2]); ptrbr = load(pre, "ptrbr", [128, 2])

            nc.vector.tensor_tensor(bias_row[:], b_ih[:], b_hh[:], op=OP.add)
            bias_glE = pre.tile([128, 2], F32, tag="bias_glE", name="bias_glE")
            nc.vector.tensor_tensor(bias_glE[:], glbr[:], glbq[:], op=OP.add)
            bias_ptrE = pre.tile([128, 2], F32, tag="bias_ptrE", name="bias_ptrE")
            nc.vector.tensor_tensor(bias_ptrE[:], ptrbr[:], ptrbq[:], op=OP.add)

            # WfT[e, o] = sum_h gl_Wr[h, e] * ptr_Wq.T[h, o]  (for F = ctx@Wf.T)
            WfT = pre.tile([128, 512], F32, tag="WfT", name="WfT")
            for m in range(2):     # e-half
                wps = preps.tile([128, 256], F32, tag="wps", name=f"wps{m}")
                for k in range(2):  # h-half
                    nc.tensor.matmul(
                        wps[:], glWr_raw[:, k * 256 + m * 128:k * 256 + m * 128 + 128],
                        ptrWqT[:, k * 256:(k + 1) * 256],
                        start=(k == 0), stop=(k == 1))
                nc.vector.tensor_copy(WfT[:, m * 256:(m + 1) * 256], wps[:])

            # bias for F: bf = ptr_Wq @ gl_br
            bfp = preps.tile([128, 2], F32, tag="bfp", name="bfp")
            for hh in range(2):
                for k in range(2):
                    nc.tensor.matmul(bfp[:, hh:hh + 1],
                                     ptrWqT[:, k * 256 + hh * 128:k * 256 + hh * 128 + 128],
                                     glbr[:, k:k + 1],
                                     start=(k == 0), stop=(k == 1))
            bias_F = pre.tile([128, 2], F32, tag="bias_F", name="bias_F")
            nc.vector.tensor_copy(bias_F[:], bfp[:])

            # stream each big tensor: mm chunk -> stage (+bias) -> transpose-repack
            def emit_packed(lhsT, biasE, dst, hl, name):
                for hh in range(2):          # output h-half
                    eps = preps.tile([128, 2048], F32, tag="eps", name=f"eps_{name}{hh}")
                    for c in range(4):
                        for k in range(2):
                            nc.tensor.matmul(
                                eps[:, c * 512:(c + 1) * 512],
                                lhsT[:, k * 256 + hh * 128:k * 256 + hh * 128 + 128],
                                ctxT[:, k * 2048 + c * 512:k * 2048 + (c + 1) * 512],
                                start=(k == 0), stop=(k == 1))
                    stage = pre.tile([128, 2048], F32, tag="stage", name=f"stg_{name}{hh}")
                    nc.vector.tensor_scalar(stage[:], in0=eps[:],
                                            scalar1=biasE[:, hh:hh + 1],
                                            scalar2=0.0, op0=OP.add)
                    st3 = stage[:].rearrange("p (b l) -> p b l", l=32)
                    for l in range(32):
                        tp = preps.tile([64, 128], F32, tag="tp", name=f"tp_{name}{hh}_{l}")
                        nc.tensor.transpose(tp[:], st3[:, :, l], ident[:])
                        if hl:
                            dstv = dst[hh * 64:(hh + 1) * 64, :] \
                                .rearrange("p (h l) -> p h l", l=32)[:, :, l]
                        else:
                            dstv = dst[hh * 64:(hh + 1) * 64, l * 128:(l + 1) * 128]
                        nc.vector.tensor_copy(dstv, tp[:])

            emit_packed(glWrT, bias_glE, E_gl, False, "gl")
            emit_packed(ptrWrT, bias_ptrE, E_ptr, False, "ptr")
            emit_packed(WfT, bias_F, F_p, True, "F")

        # ---------- working pools (opened after precompute frees space) ----------
        wk = ctx.enter_context(tc.tile_pool(name="wk", bufs=2))
        wkc = ctx.enter_context(tc.tile_pool(name="wkc", bufs=4))
        ps = ctx.enter_context(tc.tile_pool(name="ps", bufs=1, space="PSUM"))

        cur_x = st.tile([128, 128], F32, tag="xT", name="xT_init")
        nc.sync.dma_start(cur_x[:], d["xT0"][:])
        cur_h = st.tile([128, 128], F32, tag="hT", name="hT_init")   # 2*h, [h',(k,b)]
        nc.sync.dma_start(cur_h[:], d["h0T"][:])
        cur_c = st.tile([64, 256], F32, tag="cB", name="cB_init")    # [b, h]
        nc.sync.dma_start(cur_c[:], d["c0"][:])

        for t in range(L):
            # --- LSTM gates (flipped): gp[b, gdim] = x@W_ih.T + h@W_hh.T + bias
            gp = ps.tile([64, 1024], F32, tag="gp", name=f"gp{t}")
            srcs = [(cur_h, W_hhT, 0), (cur_h, W_hhT, 1),
                    (cur_x, W_ihT, 0), (cur_x, W_ihT, 1)]
            srcs = srcs[:2] + [(None, None, None)] + srcs[2:]
            for ki, (xv, Wv, kk) in enumerate(srcs):
                for c in range(2):
                    if xv is None:
                        nc.tensor.matmul(gp[:, c * 512:(c + 1) * 512],
                                         ones64[0:1, :],
                                         bias_row[0:1, c * 512:(c + 1) * 512],
                                         start=False, stop=False)
                    else:
                        nc.tensor.matmul(
                            gp[:, c * 512:(c + 1) * 512],
                            xv[:, kk * 64:(kk + 1) * 64],
                            Wv[:, kk * 1024 + c * 512:kk * 1024 + (c + 1) * 512],
                            start=(ki == 0), stop=(ki == 4))

            # sigmoid via tanh: sig(x) = 0.5*(1+tanh(x/2))
            TIF = wk.tile([64, 512], F32, tag="TIF", name=f"TIF{t}")
            nc.scalar.activation(TIF[:], gp[:, 0:512], AF.Tanh, scale=0.5)
            TG = wk.tile([64, 256], F32, tag="TG", name=f"TG{t}")
            nc.scalar.activation(TG[:], gp[:, 512:768], AF.Tanh)
            TO = wk.tile([64, 256], F32, tag="TO", name=f"TO{t}")
            nc.scalar.activation(TO[:], gp[:, 768:1024], AF.Tanh, scale=0.5)

            # c_new = 0.5*((1+tf)*c + (1+ti)*tg) ; h2 = (1+to)*tanh(c_new)
            A = wk.tile([64, 256], F32, tag="A", name=f"A{t}")
            nc.vector.scalar_tensor_tensor(A[:], in0=TIF[:, 256:512], scalar=1.0,
                                           in1=cur_c[:], op0=OP.add, op1=OP.mult)
            Bt = wk.tile([64, 256], F32, tag="Bt", name=f"Bt{t}")
            nc.vector.scalar_tensor_tensor(Bt[:], in0=TIF[:, 0:256], scalar=1.0,
                                           in1=TG[:], op0=OP.add, op1=OP.mult)
            new_c = st.tile([64, 256], F32, tag="cB", name=f"cB{t}")
            nc.vector.scalar_tensor_tensor(new_c[:], in0=A[:], scalar=0.5,
                                           in1=Bt[:], op0=OP.mult, op1=OP.add)
            TC = wk.tile([64, 256], F32, tag="TC", name=f"TC{t}")
            nc.scalar.activation(TC[:], new_c[:], AF.Tanh, scale=0.5)
            h2 = wk.tile([64, 256], F32, tag="h2", name=f"h2_{t}")
            nc.vector.scalar_tensor_tensor(h2[:], in0=TO[:], scalar=1.0,
                                           in1=TC[:], op0=OP.add, op1=OP.mult)
            cur_c = new_c

            # hT [h', (k,b)] via PE transposes (for next-step gates + qq1)
            htp = ps.tile([128, 128], F32, tag="htp", name=f"htp{t}")
            for k in range(2):
                nc.tensor.transpose(htp[:, k * 64:(k + 1) * 64],
                                    h2[:, k * 128:(k + 1) * 128], ident[0:64, 0:64])
            new_h = st.tile([128, 128], F32, tag="hT", name=f"hT{t}")
            nc.vector.tensor_copy(new_h[:], htp[:])
            cur_h = new_h

            # --- glimpse query qq1 (flipped): qq1f[b, o] = h2 @ (0.5*gl_Wq).T
            qq1f = ps.tile([64, 256], F32, tag="qq1f", name=f"qq1f{t}")
            for kk in range(2):
                nc.tensor.matmul(qq1f[:], cur_h[:, kk * 64:(kk + 1) * 64],
                                 glWqT[:, kk * 256:(kk + 1) * 256],
                                 start=(kk == 0), stop=(kk == 1))
            qq1p = wk.tile([128, 128], F32, tag="qq1p", name=f"qq1p{t}")
            nc.vector.tensor_copy(qq1p[0:64, :], qq1f[:, 0:128])
            nc.vector.tensor_copy(qq1p[64:128, :], qq1f[:, 128:256])

            # --- glimpse attention: u = sum_h v * tanh(E_gl + qq1)
            tpre = wk.tile([128, 4096], F32, tag="tpre", name=f"gtp{t}")
            for ch in range(NCH):
                sl = slice(ch * CW, (ch + 1) * CW)
                eng = nc.gpsimd if ch == 0 else nc.vector
                eng.tensor_tensor(
                    tpre[:, sl].rearrange("p (l h) -> p l h", h=128),
                    E_gl[:, sl].rearrange("p (l h) -> p l h", h=128),
                    qq1p[:].unsqueeze(1).to_broadcast([128, LC, 128]), op=OP.add)
            u2g = ps.tile([128, 32], F32, tag="u2g", name=f"u2g{t}")
            for ch in range(NCH):
                sl = slice(ch * CW, (ch + 1) * CW)
                tact = wkc.tile([128, CW], F32, tag="tact", name=f"gta{t}_{ch}")
                nc.scalar.activation(tact[:], tpre[:, sl], AF.Tanh)
                tmul = wkc.tile([128, CW], F32, tag="tmul", name=f"gtm{t}_{ch}")
                nc.vector.tensor_tensor(
                    tmul[:].rearrange("p (l h) -> p l h", h=128),
                    tact[:].rearrange("p (l h) -> p l h", h=128),
                    glv[:].unsqueeze(1).to_broadcast([128, LC, 128]), op=OP.mult)
                nc.vector.tensor_reduce(u2g[:, ch * LC:(ch + 1) * LC],
                                        tmul[:].rearrange("p (l h) -> p l h", h=128),
                                        axis=AX.X, op=OP.add)
            uglo = wk.tile([64, 32], F32, tag="uglo", name=f"uglo{t}")
            nc.vector.tensor_copy(uglo[:], u2g[0:64, :])
            ug = wk.tile([128, 32], F32, tag="ug", name=f"ug{t}")
            nc.vector.tensor_tensor(ug[0:64, :], uglo[:], u2g[64:128, :], op=OP.add)
            nc.vector.tensor_tensor(ug[64:128, :], uglo[:], u2g[64:128, :], op=OP.add)
            ugm = wk.tile([128, 32], F32, tag="ugm", name=f"ugm{t}")
            nc.vector.tensor_tensor(ugm[:], ug[:], lnmask[:], op=OP.add)

            # softmax over l (rows duplicated)
            nmax = wk.tile([128, 1], F32, tag="nmax", name=f"nmax{t}")
            nc.vector.tensor_reduce(nmax[:], ugm[:], axis=AX.X, op=OP.max, negate=True)
            exg = wk.tile([128, 32], F32, tag="exg", name=f"exg{t}")
            ssum = wk.tile([128, 1], F32, tag="ssum", name=f"ssum{t}")
            nc.scalar.activation(exg[:], ugm[:], AF.Exp, bias=nmax[:, 0:1],
                                 accum_out=ssum[:, 0:1])
            rs = wk.tile([128, 1], F32, tag="rs", name=f"rs{t}")
            nc.vector.reciprocal(rs[:], ssum[:])
            p = wk.tile([128, 32], F32, tag="p", name=f"p{t}")
            nc.vector.tensor_scalar_mul(p[:], in0=exg[:], scalar1=rs[:, 0:1])

            # --- readout: qq2 = sum_l p * F  (F packed (h', l))
            qq2p = wk.tile([128, 128], F32, tag="qq2p", name=f"qq2p{t}")
            for ch in range(NCH):
                sl = slice(ch * CW, (ch + 1) * CW)
                m2 = wkc.tile([128, CW], F32, tag="tmul", name=f"rm{t}_{ch}")
                nc.vector.tensor_tensor(
                    m2[:].rearrange("p (h l) -> p h l", l=32),
                    F_p[:, sl].rearrange("p (h l) -> p h l", l=32),
                    p[:].unsqueeze(1).to_broadcast([128, HC, 32]), op=OP.mult)
                nc.vector.tensor_reduce(qq2p[:, ch * HC:(ch + 1) * HC],
                                        m2[:].rearrange("p (h l) -> p h l", l=32),
                                        axis=AX.X, op=OP.add)

            # --- pointer attention
            # h'-chunked: chunk ch needs only qq2p[:, ch-slice] -> overlaps readout
            tpre2 = wk.tile([128, 4096], F32, tag="tpre", name=f"ptp{t}")
            Ep3 = E_ptr[:].rearrange("p (l h) -> p l h", h=128)
            tp3 = tpre2[:].rearrange("p (l h) -> p l h", h=128)
            ub_parts = {}
            for ch in range(NCH):
                hs = slice(ch * HC, (ch + 1) * HC)
                eng = nc.gpsimd if ch == 0 else nc.vector
                eng.tensor_tensor(
                    tp3[:, :, hs], Ep3[:, :, hs],
                    qq2p[:, hs].unsqueeze(1).to_broadcast([128, 32, HC]), op=OP.add)
                tact2 = wkc.tile([128, CW], F32, tag="tact", name=f"pta{t}_{ch}")
                ta3 = tact2[:].rearrange("p (l h) -> p l h", h=HC)
                nc.scalar.activation(ta3, tp3[:, :, hs], AF.Tanh)
                tmul2 = wkc.tile([128, CW], F32, tag="tmul", name=f"ptm{t}_{ch}")
                tm3 = tmul2[:].rearrange("p (l h) -> p l h", h=HC)
                nc.vector.tensor_tensor(
                    tm3, ta3,
                    ptrv[:, hs].unsqueeze(1).to_broadcast([128, 32, HC]), op=OP.mult)
                ub_parts[ch] = wkc.tile([128, 32], F32, tag="ubp", name=f"ubp{t}_{ch}")
                nc.vector.tensor_reduce(ub_parts[ch][:], tm3, axis=AX.X, op=OP.add)
            t01 = wk.tile([128, 32], F32, tag="t01", name=f"t01_{t}")
            nc.vector.tensor_tensor(t01[:], ub_parts[0][:], ub_parts[1][:], op=OP.add)
            t23 = wk.tile([128, 32], F32, tag="t23", name=f"t23_{t}")
            nc.vector.tensor_tensor(t23[:], ub_parts[2][:], ub_parts[3][:], op=OP.add)
            u2b = ps.tile([128, 32], F32, tag="u2b", name=f"u2b{t}")
            nc.vector.tensor_tensor(u2b[:], t01[:], t23[:], op=OP.add)
            uplo = wk.tile([64, 32], F32, tag="uglo", name=f"uplo{t}")
            nc.vector.tensor_copy(uplo[:], u2b[0:64, :])
            up = wk.tile([128, 32], F32, tag="up", name=f"up{t}")
            nc.vector.tensor_tensor(up[0:64, :], uplo[:], u2b[64:128, :], op=OP.add)
            nc.vector.tensor_tensor(up[64:128, :], uplo[:], u2b[64:128, :], op=OP.add)

            tl = wk.tile([128, 32], F32, tag="tl", name=f"tl{t}")
            nc.scalar.activation(tl[:], up[:], AF.Tanh)
            lgm = wk.tile([128, 32], F32, tag="lgm", name=f"lgm{t}")
            nc.vector.scalar_tensor_tensor(lgm[:], in0=tl[:], scalar=C_TANH,
                                           in1=lnmask[:], op0=OP.mult, op1=OP.add)

            # argmax (first occurrence)
            m8 = wk.tile([128, 8], F32, tag="m8", name=f"m8_{t}")
            nc.vector.max(m8[:], lgm[:])
            mi8 = wk.tile([128, 8], U32, tag="mi8", name=f"mi8_{t}")
            nc.vector.max_index(mi8[:], m8[:], lgm[:])
            selsf1 = wk.tile([128, 1], F32, tag="selsf1", name=f"selsf1_{t}")
            nc.vector.tensor_copy(selsf1[:], mi8[:, 0:1])
            nc.vector.tensor_copy(sels_f[:, t:t + 1], selsf1[0:64, :])

            onehot = wk.tile([128, 32], I32, tag="onehot", name=f"onehot{t}")
            nc.vector.tensor_scalar(onehot[:], in0=iotaL[:], scalar1=selsf1[:, 0:1],
                                    scalar2=0.0, op0=OP.is_equal)

            # deferred log_softmax pieces: lgm -> logp_buf, -max -> nmax_buf,
            # sum(exp) -> s2_buf  (ln + assembly happen once after the loop)
            nc.vector.tensor_copy(logp_buf[:, t * 32:(t + 1) * 32], lgm[0:64, :])
            nmax2 = wk.tile([128, 1], F32, tag="nmax2", name=f"nmax2_{t}")
            nc.vector.tensor_reduce(nmax2[:], lgm[:], axis=AX.X, op=OP.max, negate=True)
            nc.vector.tensor_copy(nmax_buf[:, t:t + 1], nmax2[0:64, :])
            ex2 = wk.tile([64, 32], F32, tag="ex2", name=f"ex2_{t}")
            nc.scalar.activation(ex2[:], lgm[0:64, :], AF.Exp, bias=nmax2[0:64, 0:1],
                                 accum_out=s2_buf[:, t:t + 1])

            # mask update + mask_modify
            nc.vector.copy_predicated(lnmask[:], onehot[:], minf[:])
            rmx = wk.tile([128, 1], F32, tag="rmx", name=f"rmx{t}")
            nc.vector.tensor_reduce(rmx[:], lnmask[:], axis=AX.X, op=OP.max)
            cond = wk.tile([128, 1], I32, tag="cond", name=f"cond{t}")
            nc.vector.tensor_tensor(cond[:], rmx[:], minf[:, 0:1], op=OP.is_equal)
            nc.vector.copy_predicated(lnmask[:, 31:32], cond[:], zeros1[:])

            # gather next input: x_next[b,:] = emb[idx_b, b, :]
            if t + 1 < L:
                offs = wk.tile([64, 1], I32, tag="offs", name=f"offs{t}")
                nc.vector.tensor_tensor(offs[:], iotaB32[:], mi8[0:64, 0:1], op=OP.add)
                xg = wk.tile([64, 256], F32, tag="xg", name=f"xg{t}")
                nc.gpsimd.indirect_dma_start(
                    out=xg[:], out_offset=None, in_=d["embBL"][:],
                    in_offset=bass.IndirectOffsetOnAxis(ap=offs[:, 0:1], axis=0))
                xtps = ps.tile([128, 128], F32, tag="xtps", name=f"xtps{t}")
                for k in range(2):
                    nc.tensor.transpose(xtps[:, k * 64:(k + 1) * 64],
                                        xg[:, k * 128:(k + 1) * 128],
                                        ident[0:64, 0:64])
                new_x = st.tile([128, 128], F32, tag="xT", name=f"xT{t}")
                nc.vector.tensor_copy(new_x[:], xtps[:])
                cur_x = new_x

        # ---------- deferred log_softmax finalization + outputs ----------
        lns_all = pp.tile([64, 32], F32, tag="lns_all", name="lns_all")
        nc.scalar.activation(lns_all[:], s2_buf[:], AF.Ln)
        adj = pp.tile([64, 32], F32, tag="adj", name="adj")
        nc.vector.tensor_tensor(adj[:], nmax_buf[:], lns_all[:], op=OP.subtract)
        nc.vector.tensor_tensor(
            logp_fin[:].rearrange("p (t l) -> p t l", l=32),
            logp_buf[:].rearrange("p (t l) -> p t l", l=32),
            adj[:].unsqueeze(2).to_broadcast([64, 32, 32]), op=OP.add)

        sels_i = pp.tile([64, 32], I32, tag="sels_i", name="sels_i")
        nc.vector.tensor_copy(sels_i[:], sels_f[:])
        nc.sync.dma_start(logp_o[:], logp_fin[:])
        nc.sync.dma_start(sels_o[:], sels_i[:])


# ----------------------------------------------------------------------------
# host-side shard prep
# ----------------------------------------------------------------------------
def _halves(a):
    """[256, n] -> [128, 2n] with col-block k = rows 128k:128(k+1)."""
    return np.ascontiguousarray(np.concatenate([a[0:128], a[128:256]], axis=1),
                                dtype=np.float32)


def _mask_modify_np(m):
    out = m.copy()
    allt = out.all(axis=1)
    out[allt, -1] = False
    return out


def _prep_consts(inputs):
    """Core-independent tensors (weights, biases, constants)."""
    c = {}
    c["W_ihT"] = _halves(np.ascontiguousarray(inputs["W_ih"].T))
    c["W_hhT"] = _halves(np.ascontiguousarray(inputs["W_hh"].T * 0.5))   # h stored as 2h
    c["glWqT"] = _halves(np.ascontiguousarray(inputs["gl_Wq"].T * 0.5))  # h stored as 2h
    c["ptrWqT"] = _halves(np.ascontiguousarray(inputs["ptr_Wq"].T))
    c["glWrT"] = _halves(np.ascontiguousarray(inputs["gl_Wr"].T))
    c["glWr_raw"] = _halves(np.ascontiguousarray(inputs["gl_Wr"]))
    c["ptrWrT"] = _halves(np.ascontiguousarray(inputs["ptr_Wr"].T))
    c["bias_ih"] = np.ascontiguousarray(inputs["b_ih"].reshape(1, 1024), dtype=np.float32)
    c["bias_hh"] = np.ascontiguousarray(inputs["b_hh"].reshape(1, 1024), dtype=np.float32)
    for nm, key in (("glbq", "gl_bq"), ("glbr", "gl_br"),
                    ("ptrbq", "ptr_bq"), ("ptrbr", "ptr_br")):
        c[nm] = _halves(np.ascontiguousarray(np.asarray(inputs[key]).reshape(256, 1)))
    for nm, key in (("glv_rep", "gl_v"), ("ptrv_rep", "ptr_v")):
        v = np.asarray(inputs[key], dtype=np.float32)
        rep = np.empty((128, 128), dtype=np.float32)
        rep[0:64, :] = v[0:128][None, :]
        rep[64:128, :] = v[128:256][None, :]
        c[nm] = rep
    c["ones64"] = np.ones((1, 64), dtype=np.float32)
    c["ident"] = np.eye(128, dtype=np.float32)
    c["iotaL"] = np.tile(np.arange(32, dtype=np.float32), (128, 1))
    c["iotaB32"] = (np.arange(64, dtype=np.int32) * 32).reshape(64, 1)
    c["minf"] = np.full((128, 32), NEG, dtype=np.float32)
    c["zeros1"] = np.zeros((128, 1), dtype=np.float32)
    return c


def _prep_core(inputs, consts, core):
    b0, b1 = core * BL, (core + 1) * BL
    m = dict(consts)
    m["xT0"] = _halves(np.ascontiguousarray(np.asarray(inputs["decoder_input"])[b0:b1].T))
    m["h0T"] = _halves(np.ascontiguousarray(np.asarray(inputs["h0"])[b0:b1].T * 2.0))
    m["c0"] = np.ascontiguousarray(np.asarray(inputs["c0"])[b0:b1] * 2.0,
                                   dtype=np.float32)
    ctx = np.ascontiguousarray(np.asarray(inputs["context"])[:, b0:b1, :], dtype=np.float32)
    ctxT = ctx.transpose(2, 1, 0).reshape(256, BL * L)      # [h, (b,l)]
    m["ctxTd"] = _halves(ctxT)
    emb = np.ascontiguousarray(np.asarray(inputs["embedded_inputs"])[:, b0:b1, :],
                               dtype=np.float32)
    m["embBL"] = np.ascontiguousarray(emb.transpose(1, 0, 2).reshape(BL * L, 256))
    mask0 = _mask_modify_np(np.asarray(inputs["V_reach_mask"])[b0:b1].astype(bool))
    ln = np.where(mask0, NEG, np.float32(0.0)).astype(np.float32)
    m["lnmask0"] = np.concatenate([ln, ln], axis=0)          # duplicated rows
    return m


# ----------------------------------------------------------------------------
# entry point
# ----------------------------------------------------------------------------
def kernel(**inputs):
    global _PROG
    if _PROG is None:
        _PROG = _build()
    from concourse import bass_utils
    inputs = {k: np.asarray(v) for k, v in inputs.items()}
    consts = _prep_consts(inputs)
    in_maps = [_prep_core(inputs, consts, c) for c in range(NC)]
    trace = bool(int(os.environ.get("KERNEL_TRACE", "0")))
    tkw = {}
    if trace:
        tdir = os.environ.get("KERNEL_TRACE_DIR", "/root/problem/work/trace")
        import shutil
        shutil.rmtree(tdir, ignore_errors=True)
        os.makedirs(tdir, exist_ok=True)
        tkw["tmpdir"] = tdir
    res = bass_utils.run_bass_kernel_spmd(
        _PROG, in_maps, core_ids=list(range(NC)), trace=trace, **tkw)
    if trace and res.exec_time_ns is not None:
        kernel.last_exec_ns = res.exec_time_ns
        kernel.last_profile = res.profile_json
    logp = np.concatenate([res.results[c]["logp_o"].reshape(BL, L, L)
                           for c in range(NC)], axis=0)
    sels = np.concatenate([res.results[c]["sels_o"] for c in range(NC)],
                          axis=0).astype(np.int32)
    return logp, sels


# revision 18
# speedup vs baseline: 1.2495x; 1.0051x over previous
"""TRN2 Bass kernel for nn_DeepRoute (pointer-network greedy decoder).

Self-contained: hardcodes shapes, shards batch over 8 NeuronCores,
runs one SPMD Bass program per core, gathers full outputs.

Layouts (per core, b = 64 local batch rows):
  packed attention tiles: partitions r = hh*64 + b (hh = h-half), free:
    E_gl/E_ptr: (l, h') "lh";  F: (h', l) "hl"
  LSTM state: c [64(b), 256(h)]; h kept as h2 = 2*h in [64, 256] and
  transposed hT [128(h'), (k,b)]; weights host-pre-scaled to absorb the 2x
  (sigmoid computed as 0.5*(1+tanh(x/2)) -- only tanh/exp ACT tables used).
"""
import sys, os
sys.path.insert(0, "/opt/trn_rl_repo")
import numpy as np

L, B, E, H, NC = 32, 512, 256, 256, 8
BL = B // NC          # 64 batch rows per core
C_TANH = 10.0
NEG = np.float32(-np.inf)

_PROG = None  # cached compiled Bacc program


# ----------------------------------------------------------------------------
# program builder
# ----------------------------------------------------------------------------
def _build():
    import concourse.bass as bass
    import concourse.mybir as mybir
    import concourse.tile as tile
    from concourse import bacc

    F32, I32 = mybir.dt.float32, mybir.dt.int32

    nc = bacc.Bacc("TRN2", target_bir_lowering=False, debug=False,
                   enable_asserts=False)

    d = {}
    def din(name, shape, dt=F32):
        d[name] = nc.dram_tensor(name, shape, dt, kind="ExternalInput").ap()
        return d[name]

    # per-core inputs (host-packed layouts; see _prep_core)
    din("xT0", [128, 128]); din("h0T", [128, 128]); din("c0", [64, 256])
    din("ctxTd", [128, 4096])
    din("W_ihT", [128, 2048]); din("W_hhT", [128, 2048])
    din("glWqT", [128, 512]); din("ptrWqT", [128, 512])
    din("glWrT", [128, 512]); din("ptrWrT", [128, 512]); din("glWr_raw", [128, 512])
    din("bias_ih", [1, 1024]); din("bias_hh", [1, 1024])
    din("glbq", [128, 2]); din("glbr", [128, 2])
    din("ptrbq", [128, 2]); din("ptrbr", [128, 2])
    din("glv_rep", [128, 128]); din("ptrv_rep", [128, 128])
    din("ones64", [1, 64]); din("ident", [128, 128])
    din("iotaL", [128, 32]); din("iotaB32", [64, 1], I32)
    din("minf", [128, 32]); din("zeros1", [128, 1])
    din("lnmask0", [128, 32])
    din("embBL", [BL * L, 256])

    logp_o = nc.dram_tensor("logp_o", [BL, L * L], F32, kind="ExternalOutput").ap()
    sels_o = nc.dram_tensor("sels_o", [BL, L], I32, kind="ExternalOutput").ap()

    with tile.TileContext(nc) as tc:
        _emit(tc, nc, d, logp_o, sels_o, bass, mybir)
    nc.compile()
    return nc


def _emit(tc, nc, d, logp_o, sels_o, bass, mybir):
    F32, I32, U32 = mybir.dt.float32, mybir.dt.int32, mybir.dt.uint32
    AF, OP, AX = (mybir.ActivationFunctionType, mybir.AluOpType,
                  mybir.AxisListType)
    NCH = 4                      # chunks per big DVE op
    CW = 4096 // NCH             # chunk width (cols)
    LC = 32 // NCH               # l per chunk (lh layout)
    HC = 128 // NCH              # h' per chunk (hl layout)

    from contextlib import ExitStack
    with ExitStack() as ctx:
        pp = ctx.enter_context(tc.tile_pool(name="pp", bufs=1))
        st = ctx.enter_context(tc.tile_pool(name="st", bufs=2))

        # ---------- persistent SBUF ----------
        def load(pool, name, shape, dt=F32):
            t = pool.tile(shape, dt, tag=name, name=f"sb_{name}")
            nc.sync.dma_start(t[:], d[name][:])
            return t

        W_ihT = load(pp, "W_ihT", [128, 2048])
        W_hhT = load(pp, "W_hhT", [128, 2048])
        glWqT = load(pp, "glWqT", [128, 512])
        glv = load(pp, "glv_rep", [128, 128]); ptrv = load(pp, "ptrv_rep", [128, 128])
        ones64 = load(pp, "ones64", [1, 64]); ident = load(pp, "ident", [128, 128])
        iotaL = load(pp, "iotaL", [128, 32]); iotaB32 = load(pp, "iotaB32", [64, 1], I32)
        minf = load(pp, "minf", [128, 32]); zeros1 = load(pp, "zeros1", [128, 1])

        lnmask = pp.tile([128, 32], F32, tag="lnmask", name="lnmask")
        nc.sync.dma_start(lnmask[:], d["lnmask0"][:])

        E_gl = pp.tile([128, 4096], F32, tag="E_gl", name="E_gl")     # (l, h')
        E_ptr = pp.tile([128, 4096], F32, tag="E_ptr", name="E_ptr")  # (l, h')
        F_p = pp.tile([128, 4096], F32, tag="F_p", name="F_p")        # (h', l)
        logp_buf = pp.tile([64, 1024], F32, tag="logp_buf", name="logp_buf")
        logp_fin = pp.tile([64, 1024], F32, tag="logp_fin", name="logp_fin")
        nmax_buf = pp.tile([64, 32], F32, tag="nmax_buf", name="nmax_buf")
        s2_buf = pp.tile([64, 32], F32, tag="s2_buf", name="s2_buf")
        sels_f = pp.tile([64, 32], F32, tag="sels_f", name="sels_f")
        bias_row = pp.tile([1, 1024], F32, tag="bias_row", name="bias_row")

        # ---------- precompute: E_gl, E_ptr, F (one-time) ----------
        with tc.tile_pool(name="pre", bufs=1) as pre, \
             tc.tile_pool(name="preps", bufs=1, space="PSUM") as preps:
            ctxT = load(pre, "ctxTd", [128, 4096])
            glWrT = load(pre, "glWrT", [128, 512])
            ptrWrT = load(pre, "ptrWrT", [128, 512])
            ptrWqT = load(pre, "ptrWqT", [128, 512])
            glWr_raw = load(pre, "glWr_raw", [128, 512])
            b_ih = load(pre, "bias_ih", [1, 1024])
            b_hh = load(pre, "bias_hh", [1, 1024])
            glbq = load(pre, "glbq", [128, 2]); glbr = load(pre, "glbr", [128, 2])
            ptrbq = load(pre, "ptrbq", [128, 2]); ptrbr = load(pre, "ptrbr", [128, 2])

            nc.vector.tensor_tensor(bias_row[:], b_ih[:], b_hh[:], op=OP.add)
            bias_glE = pre.tile([128, 2], F32, tag="bias_glE", name="bias_glE")
            nc.vector.tensor_tensor(bias_glE[:], glbr[:], glbq[:], op=OP.add)
            bias_ptrE = pre.tile([128, 2], F32, tag="bias_ptrE", name="bias_ptrE")
            nc.vector.tensor_tensor(bias_ptrE[:], ptrbr[:], ptrbq[:], op=OP.add)

            # WfT[e, o] = sum_h gl_Wr[h, e] * ptr_Wq.T[h, o]  (for F = ctx@Wf.T)
            WfT = pre.tile([128, 512], F32, tag="WfT", name="WfT")
            for m in range(2):     # e-half
                wps = preps.tile([128, 256], F32, tag="wps", name=f"wps{m}")
                for k in range(2):  # h-half
                    nc.tensor.matmul(
                        wps[:], glWr_raw[:, k * 256 + m * 128:k * 256 + m * 128 + 128],
                        ptrWqT[:, k * 256:(k + 1) * 256],
                        start=(k == 0), stop=(k == 1))
                nc.vector.tensor_copy(WfT[:, m * 256:(m + 1) * 256], wps[:])

            # bias for F: bf = ptr_Wq @ gl_br
            bfp = preps.tile([128, 2], F32, tag="bfp", name="bfp")
            for hh in range(2):
                for k in range(2):
                    nc.tensor.matmul(bfp[:, hh:hh + 1],
                                     ptrWqT[:, k * 256 + hh * 128:k * 256 + hh * 128 + 128],
                                     glbr[:, k:k + 1],
                                     start=(k == 0), stop=(k == 1))
            bias_F = pre.tile([128, 2], F32, tag="bias_F", name="bias_F")
            nc.vector.tensor_copy(bias_F[:], bfp[:])

            # stream each big tensor: mm chunk -> stage (+bias) -> transpose-repack
            def emit_packed(lhsT, biasE, dst, hl, name):
                for hh in range(2):          # output h-half
                    eps = preps.tile([128, 2048], F32, tag="eps", name=f"eps_{name}{hh}")
                    for c in range(4):
                        for k in range(2):
                            nc.tensor.matmul(
                                eps[:, c * 512:(c + 1) * 512],
                                lhsT[:, k * 256 + hh * 128:k * 256 + hh * 128 + 128],
                                ctxT[:, k * 2048 + c * 512:k * 2048 + (c + 1) * 512],
                                start=(k == 0), stop=(k == 1))
                    stage = pre.tile([128, 2048], F32, tag="stage", name=f"stg_{name}{hh}")
                    nc.vector.tensor_scalar(stage[:], in0=eps[:],
                                            scalar1=biasE[:, hh:hh + 1],
                                            scalar2=0.0, op0=OP.add)
                    st3 = stage[:].rearrange("p (b l) -> p b l", l=32)
                    for l in range(32):
                        tp = preps.tile([64, 128], F32, tag="tp", name=f"tp_{name}{hh}_{l}")
                        nc.tensor.transpose(tp[:], st3[:, :, l], ident[:])
                        if hl:
                            dstv = dst[hh * 64:(hh + 1) * 64, :] \
                                .rearrange("p (h l) -> p h l", l=32)[:, :, l]
                        else:
                            dstv = dst[hh * 64:(hh + 1) * 64, l * 128:(l + 1) * 128]
                        nc.vector.tensor_copy(dstv, tp[:])

            emit_packed(glWrT, bias_glE, E_gl, False, "gl")
            emit_packed(ptrWrT, bias_ptrE, E_ptr, False, "ptr")
            emit_packed(WfT, bias_F, F_p, True, "F")

        # ---------- working pools (opened after precompute frees space) ----------
        wk = ctx.enter_context(tc.tile_pool(name="wk", bufs=2))
        wkc = ctx.enter_context(tc.tile_pool(name="wkc", bufs=6))
        ps = ctx.enter_context(tc.tile_pool(name="ps", bufs=1, space="PSUM"))

        cur_x = st.tile([128, 128], F32, tag="xT", name="xT_init")
        nc.sync.dma_start(cur_x[:], d["xT0"][:])
        cur_h = st.tile([128, 128], F32, tag="hT", name="hT_init")   # 2*h, [h',(k,b)]
        nc.sync.dma_start(cur_h[:], d["h0T"][:])
        cur_c = st.tile([64, 256], F32, tag="cB", name="cB_init")    # [b, h]
        nc.sync.dma_start(cur_c[:], d["c0"][:])

        for t in range(L):
            # --- LSTM gates (flipped): gp[b, gdim] = x@W_ih.T + h@W_hh.T + bias
            gp = ps.tile([64, 1024], F32, tag="gp", name=f"gp{t}")
            srcs = [(cur_h, W_hhT, 0), (cur_h, W_hhT, 1),
                    (cur_x, W_ihT, 0), (cur_x, W_ihT, 1)]
            srcs = srcs[:2] + [(None, None, None)] + srcs[2:]
            for ki, (xv, Wv, kk) in enumerate(srcs):
                for c in range(2):
                    if xv is None:
                        nc.tensor.matmul(gp[:, c * 512:(c + 1) * 512],
                                         ones64[0:1, :],
                                         bias_row[0:1, c * 512:(c + 1) * 512],
                                         start=False, stop=False)
                    else:
                        nc.tensor.matmul(
                            gp[:, c * 512:(c + 1) * 512],
                            xv[:, kk * 64:(kk + 1) * 64],
                            Wv[:, kk * 1024 + c * 512:kk * 1024 + (c + 1) * 512],
                            start=(ki == 0), stop=(ki == 4))

            # sigmoid via tanh: sig(x) = 0.5*(1+tanh(x/2))
            TIF = wk.tile([64, 512], F32, tag="TIF", name=f"TIF{t}")
            nc.scalar.activation(TIF[:], gp[:, 0:512], AF.Tanh, scale=0.5)
            TG = wk.tile([64, 256], F32, tag="TG", name=f"TG{t}")
            nc.scalar.activation(TG[:], gp[:, 512:768], AF.Tanh)
            TO = wk.tile([64, 256], F32, tag="TO", name=f"TO{t}")
            nc.scalar.activation(TO[:], gp[:, 768:1024], AF.Tanh, scale=0.5)

            # c_new = 0.5*((1+tf)*c + (1+ti)*tg) ; h2 = (1+to)*tanh(c_new)
            A = wk.tile([64, 256], F32, tag="A", name=f"A{t}")
            nc.vector.scalar_tensor_tensor(A[:], in0=TIF[:, 256:512], scalar=1.0,
                                           in1=cur_c[:], op0=OP.add, op1=OP.mult)
            Bt = wk.tile([64, 256], F32, tag="Bt", name=f"Bt{t}")
            nc.vector.scalar_tensor_tensor(Bt[:], in0=TIF[:, 0:256], scalar=1.0,
                                           in1=TG[:], op0=OP.add, op1=OP.mult)
            new_c = st.tile([64, 256], F32, tag="cB", name=f"cB{t}")
            nc.vector.scalar_tensor_tensor(new_c[:], in0=A[:], scalar=0.5,
                                           in1=Bt[:], op0=OP.mult, op1=OP.add)
            TC = wk.tile([64, 256], F32, tag="TC", name=f"TC{t}")
            nc.scalar.activation(TC[:], new_c[:], AF.Tanh, scale=0.5)
            h2 = wk.tile([64, 256], F32, tag="h2", name=f"h2_{t}")
            nc.vector.scalar_tensor_tensor(h2[:], in0=TO[:], scalar=1.0,
                                           in1=TC[:], op0=OP.add, op1=OP.mult)
            cur_c = new_c

            # hT [h', (k,b)] via PE transposes (for next-step gates + qq1)
            htp = ps.tile([128, 128], F32, tag="htp", name=f"htp{t}")
            for k in range(2):
                nc.tensor.transpose(htp[:, k * 64:(k + 1) * 64],
                                    h2[:, k * 128:(k + 1) * 128], ident[0:64, 0:64])
            new_h = st.tile([128, 128], F32, tag="hT", name=f"hT{t}")
            nc.vector.tensor_copy(new_h[:], htp[:])
            cur_h = new_h

            # --- glimpse query qq1 (flipped): qq1f[b, o] = h2 @ (0.5*gl_Wq).T
            qq1f = ps.tile([64, 256], F32, tag="qq1f", name=f"qq1f{t}")
            for kk in range(2):
                nc.tensor.matmul(qq1f[:], cur_h[:, kk * 64:(kk + 1) * 64],
                                 glWqT[:, kk * 256:(kk + 1) * 256],
                                 start=(kk == 0), stop=(kk == 1))
            qq1p = wk.tile([128, 128], F32, tag="qq1p", name=f"qq1p{t}")
            nc.vector.tensor_copy(qq1p[0:64, :], qq1f[:, 0:128])
            nc.vector.tensor_copy(qq1p[64:128, :], qq1f[:, 128:256])

            # --- glimpse attention: u = sum_h v * tanh(E_gl + qq1)
            tpre = wk.tile([128, 4096], F32, tag="tpre", name=f"gtp{t}")
            for ch in range(NCH):
                sl = slice(ch * CW, (ch + 1) * CW)
                eng = nc.gpsimd if ch == 0 else nc.vector
                eng.tensor_tensor(
                    tpre[:, sl].rearrange("p (l h) -> p l h", h=128),
                    E_gl[:, sl].rearrange("p (l h) -> p l h", h=128),
                    qq1p[:].unsqueeze(1).to_broadcast([128, LC, 128]), op=OP.add)
            u2g = ps.tile([128, 32], F32, tag="u2g", name=f"u2g{t}")
            for ch in range(NCH):
                sl = slice(ch * CW, (ch + 1) * CW)
                tact = wkc.tile([128, CW], F32, tag="tact", name=f"gta{t}_{ch}")
                nc.scalar.activation(tact[:], tpre[:, sl], AF.Tanh)
                tmul = wkc.tile([128, CW], F32, tag="tmul", name=f"gtm{t}_{ch}")
                nc.vector.tensor_tensor(
                    tmul[:].rearrange("p (l h) -> p l h", h=128),
                    tact[:].rearrange("p (l h) -> p l h", h=128),
                    glv[:].unsqueeze(1).to_broadcast([128, LC, 128]), op=OP.mult)
                nc.vector.tensor_reduce(u2g[:, ch * LC:(ch + 1) * LC],
                                        tmul[:].rearrange("p (l h) -> p l h", h=128),
                                        axis=AX.X, op=OP.add)
            uglo = wk.tile([64, 32], F32, tag="uglo", name=f"uglo{t}")
            nc.vector.tensor_copy(uglo[:], u2g[0:64, :])
            ug = wk.tile([128, 32], F32, tag="ug", name=f"ug{t}")
            nc.vector.tensor_tensor(ug[0:64, :], uglo[:], u2g[64:128, :], op=OP.add)
            nc.vector.tensor_tensor(ug[64:128, :], uglo[:], u2g[64:128, :], op=OP.add)
            ugm = wk.tile([128, 32], F32, tag="ugm", name=f"ugm{t}")
            nc.vector.tensor_tensor(ugm[:], ug[:], lnmask[:], op=OP.add)

            # softmax over l (rows duplicated)
            nmax = wk.tile([128, 1], F32, tag="nmax", name=f"nmax{t}")
            nc.vector.tensor_reduce(nmax[:], ugm[:], axis=AX.X, op=OP.max, negate=True)
            exg = wk.tile([128, 32], F32, tag="exg", name=f"exg{t}")
            ssum = wk.tile([128, 1], F32, tag="ssum", name=f"ssum{t}")
            nc.scalar.activation(exg[:], ugm[:], AF.Exp, bias=nmax[:, 0:1],
                                 accum_out=ssum[:, 0:1])
            rs = wk.tile([128, 1], F32, tag="rs", name=f"rs{t}")
            nc.vector.reciprocal(rs[:], ssum[:])
            p = wk.tile([128, 32], F32, tag="p", name=f"p{t}")
            nc.vector.tensor_scalar_mul(p[:], in0=exg[:], scalar1=rs[:, 0:1])

            # --- readout: qq2 = sum_l p * F  (F packed (h', l))
            qq2p = wk.tile([128, 128], F32, tag="qq2p", name=f"qq2p{t}")
            for ch in range(NCH):
                sl = slice(ch * CW, (ch + 1) * CW)
                m2 = wkc.tile([128, CW], F32, tag="tmul", name=f"rm{t}_{ch}")
                nc.vector.tensor_tensor(
                    m2[:].rearrange("p (h l) -> p h l", l=32),
                    F_p[:, sl].rearrange("p (h l) -> p h l", l=32),
                    p[:].unsqueeze(1).to_broadcast([128, HC, 32]), op=OP.mult)
                nc.vector.tensor_reduce(qq2p[:, ch * HC:(ch + 1) * HC],
                                        m2[:].rearrange("p (h l) -> p h l", l=32),
                                        axis=AX.X, op=OP.add)

            # --- pointer attention
            # h'-chunked: chunk ch needs only qq2p[:, ch-slice] -> overlaps readout
            tpre2 = wk.tile([128, 4096], F32, tag="tpre", name=f"ptp{t}")
            Ep3 = E_ptr[:].rearrange("p (l h) -> p l h", h=128)
            tp3 = tpre2[:].rearrange("p (l h) -> p l h", h=128)
            ub_parts = {}
            for ch in range(NCH):
                hs = slice(ch * HC, (ch + 1) * HC)
                eng = nc.gpsimd if ch == 0 else nc.vector
                eng.tensor_tensor(
                    tp3[:, :, hs], Ep3[:, :, hs],
                    qq2p[:, hs].unsqueeze(1).to_broadcast([128, 32, HC]), op=OP.add)
                tact2 = wkc.tile([128, CW], F32, tag="tact", name=f"pta{t}_{ch}")
                ta3 = tact2[:].rearrange("p (l h) -> p l h", h=HC)
                nc.scalar.activation(ta3, tp3[:, :, hs], AF.Tanh)
                tmul2 = wkc.tile([128, CW], F32, tag="tmul", name=f"ptm{t}_{ch}")
                tm3 = tmul2[:].rearrange("p (l h) -> p l h", h=HC)
                nc.vector.tensor_tensor(
                    tm3, ta3,
                    ptrv[:, hs].unsqueeze(1).to_broadcast([128, 32, HC]), op=OP.mult)
                ub_parts[ch] = wkc.tile([128, 32], F32, tag="ubp", name=f"ubp{t}_{ch}")
                nc.vector.tensor_reduce(ub_parts[ch][:], tm3, axis=AX.X, op=OP.add)
            t01 = wk.tile([128, 32], F32, tag="t01", name=f"t01_{t}")
            nc.vector.tensor_tensor(t01[:], ub_parts[0][:], ub_parts[1][:], op=OP.add)
            t23 = wk.tile([128, 32], F32, tag="t23", name=f"t23_{t}")
            nc.vector.tensor_tensor(t23[:], ub_parts[2][:], ub_parts[3][:], op=OP.add)
            u2b = ps.tile([128, 32], F32, tag="u2b", name=f"u2b{t}")
            nc.vector.tensor_tensor(u2b[:], t01[:], t23[:], op=OP.add)
            uplo = wk.tile([64, 32], F32, tag="uglo", name=f"uplo{t}")
            nc.vector.tensor_copy(uplo[:], u2b[0:64, :])
            up = wk.tile([128, 32], F32, tag="up", name=f"up{t}")
            nc.vector.tensor_tensor(up[0:64, :], uplo[:], u2b[64:128, :], op=OP.add)
            nc.vector.tensor_tensor(up[64:128, :], uplo[:], u2b[64:128, :], op=OP.add)

            tl = wk.tile([128, 32], F32, tag="tl", name=f"tl{t}")
            nc.scalar.activation(tl[:], up[:], AF.Tanh)
            lgm = wk.tile([128, 32], F32, tag="lgm", name=f"lgm{t}")
            nc.vector.scalar_tensor_tensor(lgm[:], in0=tl[:], scalar=C_TANH,
                                           in1=lnmask[:], op0=OP.mult, op1=OP.add)

            # argmax (first occurrence)
            m8 = wk.tile([128, 8], F32, tag="m8", name=f"m8_{t}")
            nc.vector.max(m8[:], lgm[:])
            mi8 = wk.tile([128, 8], U32, tag="mi8", name=f"mi8_{t}")
            nc.vector.max_index(mi8[:], m8[:], lgm[:])
            selsf1 = wk.tile([128, 1], F32, tag="selsf1", name=f"selsf1_{t}")
            nc.vector.tensor_copy(selsf1[:], mi8[:, 0:1])
            nc.vector.tensor_copy(sels_f[:, t:t + 1], selsf1[0:64, :])

            onehot = wk.tile([128, 32], I32, tag="onehot", name=f"onehot{t}")
            nc.vector.tensor_scalar(onehot[:], in0=iotaL[:], scalar1=selsf1[:, 0:1],
                                    scalar2=0.0, op0=OP.is_equal)

            # deferred log_softmax pieces: lgm -> logp_buf, -max -> nmax_buf,
            # sum(exp) -> s2_buf  (ln + assembly happen once after the loop)
            nc.vector.tensor_copy(logp_buf[:, t * 32:(t + 1) * 32], lgm[0:64, :])
            nmax2 = wk.tile([128, 1], F32, tag="nmax2", name=f"nmax2_{t}")
            nc.vector.tensor_reduce(nmax2[:], lgm[:], axis=AX.X, op=OP.max, negate=True)
            nc.vector.tensor_copy(nmax_buf[:, t:t + 1], nmax2[0:64, :])
            ex2 = wk.tile([64, 32], F32, tag="ex2", name=f"ex2_{t}")
            nc.scalar.activation(ex2[:], lgm[0:64, :], AF.Exp, bias=nmax2[0:64, 0:1],
                                 accum_out=s2_buf[:, t:t + 1])

            # mask update + mask_modify
            nc.vector.copy_predicated(lnmask[:], onehot[:], minf[:])
            rmx = wk.tile([128, 1], F32, tag="rmx", name=f"rmx{t}")
            nc.vector.tensor_reduce(rmx[:], lnmask[:], axis=AX.X, op=OP.max)
            cond = wk.tile([128, 1], I32, tag="cond", name=f"cond{t}")
            nc.vector.tensor_tensor(cond[:], rmx[:], minf[:, 0:1], op=OP.is_equal)
            nc.vector.copy_predicated(lnmask[:, 31:32], cond[:], zeros1[:])

            # gather next input: x_next[b,:] = emb[idx_b, b, :]
            if t + 1 < L:
                offs = wk.tile([64, 1], I32, tag="offs", name=f"offs{t}")
                nc.vector.tensor_tensor(offs[:], iotaB32[:], mi8[0:64, 0:1], op=OP.add)
                xg = wk.tile([64, 256], F32, tag="xg", name=f"xg{t}")
                nc.gpsimd.indirect_dma_start(
                    out=xg[:], out_offset=None, in_=d["embBL"][:],
                    in_offset=bass.IndirectOffsetOnAxis(ap=offs[:, 0:1], axis=0))
                xtps = ps.tile([128, 128], F32, tag="xtps", name=f"xtps{t}")
                for k in range(2):
                    nc.tensor.transpose(xtps[:, k * 64:(k + 1) * 64],
                                        xg[:, k * 128:(k + 1) * 128],
                                        ident[0:64, 0:64])
                new_x = st.tile([128, 128], F32, tag="xT", name=f"xT{t}")
                nc.vector.tensor_copy(new_x[:], xtps[:])
                cur_x = new_x

        # ---------- deferred log_softmax finalization + outputs ----------
        lns_all = pp.tile([64, 32], F32, tag="lns_all", name="lns_all")
        nc.scalar.activation(lns_all[:], s2_buf[:], AF.Ln)
        adj = pp.tile([64, 32], F32, tag="adj", name="adj")
        nc.vector.tensor_tensor(adj[:], nmax_buf[:], lns_all[:], op=OP.subtract)
        nc.vector.tensor_tensor(
            logp_fin[:].rearrange("p (t l) -> p t l", l=32),
            logp_buf[:].rearrange("p (t l) -> p t l", l=32),
            adj[:].unsqueeze(2).to_broadcast([64, 32, 32]), op=OP.add)

        sels_i = pp.tile([64, 32], I32, tag="sels_i", name="sels_i")
        nc.vector.tensor_copy(sels_i[:], sels_f[:])
        nc.sync.dma_start(logp_o[:], logp_fin[:])
        nc.sync.dma_start(sels_o[:], sels_i[:])


# ----------------------------------------------------------------------------
# host-side shard prep
# ----------------------------------------------------------------------------
def _halves(a):
    """[256, n] -> [128, 2n] with col-block k = rows 128k:128(k+1)."""
    return np.ascontiguousarray(np.concatenate([a[0:128], a[128:256]], axis=1),
                                dtype=np.float32)


def _mask_modify_np(m):
    out = m.copy()
    allt = out.all(axis=1)
    out[allt, -1] = False
    return out


def _prep_consts(inputs):
    """Core-independent tensors (weights, biases, constants)."""
    c = {}
    c["W_ihT"] = _halves(np.ascontiguousarray(inputs["W_ih"].T))
    c["W_hhT"] = _halves(np.ascontiguousarray(inputs["W_hh"].T * 0.5))   # h stored as 2h
    c["glWqT"] = _halves(np.ascontiguousarray(inputs["gl_Wq"].T * 0.5))  # h stored as 2h
    c["ptrWqT"] = _halves(np.ascontiguousarray(inputs["ptr_Wq"].T))
    c["glWrT"] = _halves(np.ascontiguousarray(inputs["gl_Wr"].T))
    c["glWr_raw"] = _halves(np.ascontiguousarray(inputs["gl_Wr"]))
    c["ptrWrT"] = _halves(np.ascontiguousarray(inputs["ptr_Wr"].T))
    c["bias_ih"] = np.ascontiguousarray(inputs["b_ih"].reshape(1, 1024), dtype=np.float32)
    c["bias_hh"] = np.ascontiguousarray(inputs["b_hh"].reshape(1, 1024), dtype=np.float32)
    for nm, key in (("glbq", "gl_bq"), ("glbr", "gl_br"),
                    ("ptrbq", "ptr_bq"), ("ptrbr", "ptr_br")):
        c[nm] = _halves(np.ascontiguousarray(np.asarray(inputs[key]).reshape(256, 1)))
    for nm, key in (("glv_rep", "gl_v"), ("ptrv_rep", "ptr_v")):
        v = np.asarray(inputs[key], dtype=np.float32)
        rep = np.empty((128, 128), dtype=np.float32)
        rep[0:64, :] = v[0:128][None, :]
        rep[64:128, :] = v[128:256][None, :]
        c[nm] = rep
    c["ones64"] = np.ones((1, 64), dtype=np.float32)
    c["ident"] = np.eye(128, dtype=np.float32)
    c["iotaL"] = np.tile(np.arange(32, dtype=np.float32), (128, 1))
    c["iotaB32"] = (np.arange(64, dtype=np.int32) * 32).reshape(64, 1)
    c["minf"] = np.full((128, 32), NEG, dtype=np.float32)
    c["zeros1"] = np.zeros((128, 1), dtype=np.float32)
    return c


def _prep_core(inputs, consts, core):
    b0, b1 = core * BL, (core + 1) * BL
    m = dict(consts)
    m["xT0"] = _halves(np.ascontiguousarray(np.asarray(inputs["decoder_input"])[b0:b1].T))
    m["h0T"] = _halves(np.ascontiguousarray(np.asarray(inputs["h0"])[b0:b1].T * 2.0))
    m["c0"] = np.ascontiguousarray(np.asarray(inputs["c0"])[b0:b1] * 2.0,
                                   dtype=np.float32)
    ctx = np.ascontiguousarray(np.asarray(inputs["context"])[:, b0:b1, :], dtype=np.float32)
    ctxT = ctx.transpose(2, 1, 0).reshape(256, BL * L)      # [h, (b,l)]
    m["ctxTd"] = _halves(ctxT)
    emb = np.ascontiguousarray(np.asarray(inputs["embedded_inputs"])[:, b0:b1, :],
                               dtype=np.float32)
    m["embBL"] = np.ascontiguousarray(emb.transpose(1, 0, 2).reshape(BL * L, 256))
    mask0 = _mask_modify_np(np.asarray(inputs["V_reach_mask"])[b0:b1].astype(bool))
    ln = np.where(mask0, NEG, np.float32(0.0)).astype(np.float32)
    m["lnmask0"] = np.concatenate([ln, ln], axis=0)          # duplicated rows
    return m


# ----------------------------------------------------------------------------
# entry point
# ----------------------------------------------------------------------------
def kernel(**inputs):
    global _PROG
    if _PROG is None:
        _PROG = _build()
    from concourse import bass_utils
    inputs = {k: np.asarray(v) for k, v in inputs.items()}
    consts = _prep_consts(inputs)
    in_maps = [_prep_core(inputs, consts, c) for c in range(NC)]
    trace = bool(int(os.environ.get("KERNEL_TRACE", "0")))
    tkw = {}
    if trace:
        tdir = os.environ.get("KERNEL_TRACE_DIR", "/root/problem/work/trace")
        import shutil
        shutil.rmtree(tdir, ignore_errors=True)
        os.makedirs(tdir, exist_ok=True)
        tkw["tmpdir"] = tdir
    res = bass_utils.run_bass_kernel_spmd(
        _PROG, in_maps, core_ids=list(range(NC)), trace=trace, **tkw)
    if trace and res.exec_time_ns is not None:
        kernel.last_exec_ns = res.exec_time_ns
        kernel.last_profile = res.profile_json
    logp = np.concatenate([res.results[c]["logp_o"].reshape(BL, L, L)
                           for c in range(NC)], axis=0)
    sels = np.concatenate([res.results[c]["sels_o"] for c in range(NC)],
                          axis=0).astype(np.int32)
    return logp, sels
